# revision 12
# baseline (speedup 1.0000x reference)
"""nn_InteractionLayer Bass/Tile kernel for 8 Trainium2 NeuronCores.

out = where(dist < 1, exp(-2*(1/dist - 1)^2), 0) @ (z @ W + B)
N = 8192, D = 256.

Row-parallel: core c owns rows [c*1024, (c+1)*1024) of dist_matrix.
dist is shipped host-side as fp16 (halves the dominant HBM read) in
transposed [j, i] layout; a host boundary fix keeps the r<1 cutoff
bit-exact across the fp16 rounding. z is shipped transposed+fp16.

Per-core dataflow, 16 chunks of [128, 4096] (4 j-tiles):
  msg   = z @ (W*sqrt(pi)/2)  [N, D] fp16 via 16 PSUM batches (bias
          matmuls only if B is nonzero; B is zero in this problem).
          The sqrt(pi)/2 pre-scale host-side cancels Derivative_Erf's
          2/sqrt(pi) factor.
  m     = (r >= 1) * 60000          DVE ts chain (4x mode)
  t     = 1/r:  ACT raw Reciprocal for chunks in ACT_RECIP, else a
          DVE stock-op Newton chain (bitwise-NOT seed on the fp16 bit
          pattern + 1 Newton step; ~2.6e-3 rel err, plenty for the
          2e-2 gate) to offload the saturated ACT engine.
  t'    = max(t, m)                 DVE tt (masked elems -> 60000)
  w     = Derivative_Erf(sqrt2*t' - sqrt2) = (2/sqrt(pi))*exp(-2(t'-1)^2)
          ACT, immediate scale/bias; masked input saturates to exact 0.
  outT[d, i] += msg_chunk^T @ w     PE, PSUM fp32, 4 banks

ACT table sets are batched per 4-chunk super (recips then previous
super's D_Erfs) to bound table reloads. Reciprocal/Derivative_Erf are
emitted as raw InstActivation (wrapper vetoes Reciprocal on accuracy
grounds; measured ~1e-5 rel here, tolerance 2e-2).

This container's walrus encodes at most ONE semaphore wait per TPB
instruction; a post-Tile pass splits extra waits onto same-engine
EventSemaphore carriers (semantically identical, program order).
"""

import sys
import types

if "/opt/trn_rl_repo" not in sys.path:
    sys.path.insert(0, "/opt/trn_rl_repo")

import numpy as np

N = 8192
D = 256
NCORES = 8
ROWS = N // NCORES  # 1024 rows of dist per core
JT = 128  # j-tile (partition dim)
NJT = N // JT  # 64 j-tiles
CHUNK_JT = 4  # j-tiles per elementwise chunk
CHUNK_F = CHUNK_JT * ROWS  # free-dim elements per chunk tile (4096)
NCHUNK = NJT // CHUNK_JT  # 16
K = 4  # chunks per superchunk (ACT table-set batch)
NSUPER = NCHUNK // K  # 4

# chunks whose reciprocal runs on DVE (Newton) instead of ACT; grouped so
# super 1 has no ACT recips at all (keeps the D_Erf table resident)
DVE_RECIP = frozenset({4, 5, 6, 7, 13})
# msg PSUM->SBUF copy engine per batch: True -> ACT, False -> DVE
MSG_ON_ACT = tuple(False for _ in range(NCHUNK))

SQ2 = 1.4142135623730951
WSCALE = 0.8862269254527580  # sqrt(pi)/2, cancels D_Erf's 2/sqrt(pi)
# fp16 bitwise-NOT reciprocal seed + 1 Newton: y1 = c0*s*(c1 - r*c0*s),
# s = bitcast16(~bits16(r)). Constants minimax-fit over [0.05, 2.05].
NR_C0 = -0.23563272
NR_C1 = 2.00172757

_CACHE = {}


def _install_ntff_hook():
    """Provide antenv.axon_hooks (absent in this image) so trace=True can
    NTFF-profile through libaxon. Only needed for profiling runs."""
    if "antenv.axon_hooks" in sys.modules:
        return
    import antenv

    mod = types.ModuleType("antenv.axon_hooks")
    state = {"hook": None}
    mod.set_axon_ntff_profile_hook = lambda h: state.__setitem__("hook", h)
    mod.get_axon_ntff_profile_hook = lambda: state["hook"]
    sys.modules["antenv.axon_hooks"] = mod
    antenv.axon_hooks = mod
    try:
        from trn_agent_boot.trn_boot import _ntff_profile_via_ctypes

        mod.set_axon_ntff_profile_hook(
            _ntff_profile_via_ctypes("/opt/axon/libaxon_pjrt.so")
        )
    except Exception:
        pass


def _split_excess_waits(nc, max_waits=1):
    """Walrus here encodes at most one sync-wait per TPB instruction.
    Hoist extras onto preceding same-engine wait-only carriers."""
    import bass_rust

    seq = 0
    for fn in nc.m.functions:
        for bb in fn.blocks:
            insts = list(bb.instructions)
            out = []
            dirty = False
            for inst in insts:
                si = inst.sync_info
                if si is None:
                    out.append(inst)
                    continue
                waits = list(si.on_wait)
                if len(waits) > max_waits:
                    for w in waits[:-max_waits]:
                        seq += 1
                        carrier = bass_rust.InstEventSemaphore(
                            name=f"WSPLIT-{seq}", ins=[], outs=[]
                        )
                        carrier.engine = inst.engine
                        carrier.sync_info = bass_rust.SyncInfo(
                            on_wait=[w], on_update=[]
                        )
                        out.append(carrier)
                    inst.sync_info = bass_rust.SyncInfo(
                        on_wait=waits[-max_waits:], on_update=list(si.on_update)
                    )
                    dirty = True
                out.append(inst)
            if dirty:
                bb.instructions = out
    return seq


def _build(has_bias):
    import concourse.bass as bass
    import concourse.tile as tile
    from concourse import mybir

    f32 = mybir.dt.float32
    f16 = mybir.dt.float16
    u16 = mybir.dt.uint16
    AF = mybir.ActivationFunctionType
    OP = mybir.AluOpType

    nc = bass.Bass(
        "TRN2", target_bir_lowering=False, debug=False, num_devices=NCORES
    )
    distT_d = nc.dram_tensor("distT", [N, ROWS], f16, kind="ExternalInput").ap()
    zT_d = nc.dram_tensor("zT", [D, N], f16, kind="ExternalInput").ap()
    w_d = nc.dram_tensor("w", [D, D], f16, kind="ExternalInput").ap()
    b_d = nc.dram_tensor("b", [1, D], f16, kind="ExternalInput").ap()
    outT_d = nc.dram_tensor("outT", [D, ROWS], f32, kind="ExternalOutput").ap()

    def act_raw(out_ap, in_ap, func, bias, scale):
        return nc.scalar.add_instruction(
            mybir.InstActivation(
                name=nc.get_next_instruction_name(),
                func=func,
                ins=[
                    nc.scalar.lower_ap(in_ap),
                    mybir.ImmediateValue(dtype=f32, value=bias),
                    mybir.ImmediateValue(dtype=f32, value=scale),
                    mybir.ImmediateValue(dtype=f32, value=0.0),
                ],
                outs=[nc.scalar.lower_ap(out_ap)],
            )
        )

    with tile.TileContext(nc) as tc:
        with (
            tc.tile_pool(name="wb", bufs=1) as wbpool,
            tc.tile_pool(name="zq", bufs=1) as zqpool,
            tc.tile_pool(name="msgp", bufs=1) as msgpool,
            tc.tile_pool(name="msgps", bufs=2, space="PSUM") as msgpsum,
            tc.tile_pool(name="rch", bufs=3) as rpool,
            tc.tile_pool(name="mch", bufs=4) as mpool,
            tc.tile_pool(name="tch", bufs=5) as tpool,
            tc.tile_pool(name="nsc", bufs=1) as nscpool,
            tc.tile_pool(name="wch", bufs=2) as wpool,
            tc.tile_pool(name="outps", bufs=1, space="PSUM") as outpsum,
            tc.tile_pool(name="outsb", bufs=1) as outpool,
        ):
            # ---- W (fp16, host-scaled), optional bias, zT halves ----
            w_sb = [
                wbpool.tile([JT, D], f16, tag=f"wsb{h}", name=f"wsb{h}")
                for h in (0, 1)
            ]
            zq = [
                zqpool.tile([JT, N], f16, tag=f"zq{h}", name=f"zq{h}")
                for h in (0, 1)
            ]

            def load_z_quarter(q):
                for h in (0, 1):
                    nc.sync.dma_start(
                        zq[h][:, q * 2048 : (q + 1) * 2048],
                        zT_d[h * JT : (h + 1) * JT, q * 2048 : (q + 1) * 2048],
                    )

            def load_wb():
                for h in (0, 1):
                    nc.sync.dma_start(w_sb[h][:], w_d[h * JT : (h + 1) * JT, :])
                if has_bias:
                    nc.sync.dma_start(b_sb[:], b_d[:])
                    nc.gpsimd.memset(ones[:], 1.0)

            if has_bias:
                b_sb = wbpool.tile([1, D], f16)
                ones = wbpool.tile([1, JT], f16)

            msg = [
                msgpool.tile([JT, CHUNK_JT * D], f16, name=f"msg{b}", tag=f"msg{b}")
                for b in range(NCHUNK)
            ]
            acc = [
                outpsum.tile([JT, ROWS], f32, tag=f"acc{h}", name=f"acc{h}")
                for h in (0, 1)
            ]

            def emit_msg_batch(B):
                ps = msgpsum.tile([JT, CHUNK_JT * D], f32, name=f"mps{B}", tag="mps")
                for jj in range(CHUNK_JT):
                    jg = B * CHUNK_JT + jj
                    pslice = ps[:, jj * D : (jj + 1) * D]
                    for h in (0, 1):
                        nc.tensor.matmul(
                            pslice,
                            zq[h][:, jg * JT : (jg + 1) * JT],
                            w_sb[h][:],
                            start=(h == 0),
                            stop=(h == 1 and not has_bias),
                        )
                    if has_bias:
                        nc.tensor.matmul(
                            pslice, ones[:], b_sb[:], start=False, stop=True
                        )
                if MSG_ON_ACT[B]:
                    nc.scalar.copy(msg[B][:], ps[:])
                else:
                    nc.vector.tensor_copy(msg[B][:], ps[:])

            def emit_r_dma(c):
                r = rpool.tile([JT, CHUNK_F], f16, name=f"r{c}", tag="r")
                for k in range(CHUNK_JT):
                    jt = c * CHUNK_JT + k
                    nc.sync.dma_start(
                        r[:, k * ROWS : (k + 1) * ROWS],
                        distT_d[jt * JT : (jt + 1) * JT, :],
                    )
                return r

            def emit_phase1(c, r):
                m = mpool.tile([JT, CHUNK_F], f16, name=f"m{c}", tag="m")
                nc.vector.tensor_scalar(
                    m[:], r[:], 1.0, 60000.0, op0=OP.is_ge, op1=OP.mult
                )
                t = tpool.tile([JT, CHUNK_F], f16, name=f"t{c}", tag="t")
                if c in DVE_RECIP:
                    # s = bitcast(~r); p = r*s; q = (p - c1/c0)*(-c0^2);
                    # t = s*q  ==  c0*s*(c1 - r*c0*s)
                    s_t = nscpool.tile([JT, CHUNK_F], u16, name=f"ns{c}", tag="ns")
                    nc.vector.tensor_scalar(
                        s_t[:], r[:].bitcast(u16), 0, None, op0=OP.bitwise_not
                    )
                    p_t = nscpool.tile([JT, CHUNK_F], f16, name=f"np{c}", tag="np")
                    nc.vector.tensor_tensor(
                        p_t[:], r[:], s_t[:].bitcast(f16), op=OP.mult
                    )
                    nc.vector.tensor_scalar(
                        p_t[:], p_t[:], NR_C1 / NR_C0, -NR_C0 * NR_C0,
                        op0=OP.subtract, op1=OP.mult,
                    )
                    nc.vector.tensor_tensor(
                        t[:], s_t[:].bitcast(f16), p_t[:], op=OP.mult
                    )
                else:
                    act_raw(t[:], r[:], AF.Reciprocal, 0.0, 1.0)
                return c, t, m

            def emit_apply(c, t, m):
                # masked elements -> 60000; D_Erf saturates them to exact 0
                nc.vector.tensor_tensor(t[:], t[:], m[:], op=OP.max)

            def emit_derf_pe(c, t):
                w = wpool.tile([JT, CHUNK_F], f16, name=f"w{c}", tag="w")
                act_raw(w[:], t[:], AF.Derivative_Erf, -SQ2, SQ2)
                for k in range(CHUNK_JT):
                    jt = c * CHUNK_JT + k
                    mtile = msg[c]
                    for h in (0, 1):
                        lhsT = mtile[:, k * D + h * JT : k * D + (h + 1) * JT]
                        for nh in (0, 1):
                            nc.tensor.matmul(
                                acc[h][:, nh * 512 : (nh + 1) * 512],
                                lhsT,
                                w[:, k * ROWS + nh * 512 : k * ROWS + (nh + 1) * 512],
                                start=(jt == 0),
                                stop=(jt == NJT - 1),
                            )

            # ---- interleaved emission ----
            # Per-engine program order is what the in-order engines execute:
            #   DVE: applies(s) | msg casts(s+1) | makes+newtons(s+1)
            #   ACT: derfs(s)   | recips(s+1)
            # so a super's phase-2 never queues behind the next super's
            # phase-1 on either engine.
            rt = {c: emit_r_dma(c) for c in range(K)}
            load_wb()
            load_z_quarter(0)
            pending = [emit_phase1(c, rt.pop(c)) for c in range(K)]
            for c in range(K, 2 * K):
                rt[c] = emit_r_dma(c)
            for B in range(K):
                emit_msg_batch(B)
            for s in range(NSUPER):
                applied = []
                for c, t, m in pending:
                    emit_apply(c, t, m)
                    applied.append((c, t))
                if s + 1 < NSUPER:
                    load_z_quarter(s + 1)
                    for B in range(K * (s + 1), K * (s + 1) + K):
                        emit_msg_batch(B)
                for c, t in applied:
                    emit_derf_pe(c, t)
                if s + 2 < NSUPER:
                    for c in range(K * (s + 2), K * (s + 2) + K):
                        rt[c] = emit_r_dma(c)
                if s + 1 < NSUPER:
                    pending = [
                        emit_phase1(c, rt.pop(c))
                        for c in range(K * (s + 1), K * (s + 1) + K)
                    ]

            # ---- tail: PSUM -> SBUF fp32 -> HBM ----
            for h in (0, 1):
                o = outpool.tile([JT, ROWS], f32, tag=f"o{h}", name=f"o{h}")
                nc.vector.tensor_copy(o[:], acc[h][:])
                nc.sync.dma_start(outT_d[h * JT : (h + 1) * JT, :], o[:])

    _split_excess_waits(nc)
    return nc


def kernel(z, dist_matrix, W, B, _trace=False):
    from concourse.bass_utils import run_bass_kernel_spmd

    if _trace:
        _install_ntff_hook()

    dist = np.asarray(dist_matrix, np.float32)
    z = np.asarray(z, np.float32)
    W_np = np.asarray(W, np.float32)
    B_np = np.asarray(B, np.float32).reshape(1, D)
    has_bias = bool(np.any(B_np))

    key = ("nc", has_bias)
    if key not in _CACHE:
        _CACHE[key] = _build(has_bias)
    nc = _CACHE[key]

    # fp16 dist with an exact cutoff: values < 1 that round UP to 1.0
    # would flip the mask; pin them to the largest fp16 below 1.
    r16 = dist.astype(np.float16)
    bad = (dist < 1.0) & (r16 >= 1.0)
    if bad.any():
        r16[bad] = np.float16(0.99951171875)

    zT16 = np.ascontiguousarray(z.T.astype(np.float16))
    W16 = (W_np * WSCALE).astype(np.float16)
    B16 = (B_np * WSCALE).astype(np.float16)

    in_maps = []
    for c in range(NCORES):
        blk = np.ascontiguousarray(r16[c * ROWS : (c + 1) * ROWS, :].T)
        in_maps.append({"distT": blk, "zT": zT16, "w": W16, "b": B16})

    res = run_bass_kernel_spmd(
        nc, in_maps, core_ids=list(range(NCORES)), trace=_trace
    )
    _CACHE["last"] = res

    out = np.empty((N, D), np.float32)
    for c in range(NCORES):
        out[c * ROWS : (c + 1) * ROWS, :] = res.results[c]["outT"].T
    return out


# revision 20
# speedup vs baseline: 1.1805x; 1.1805x over previous
"""nn_InteractionLayer Bass/Tile kernel for 8 Trainium2 NeuronCores.

out = where(dist < 1, exp(-2*(1/dist - 1)^2), 0) @ (z @ W + B)
N = 8192, D = 256.

Row-parallel: core c owns rows [c*1024, (c+1)*1024) of dist_matrix.
dist is shipped host-side as fp16 (halves the dominant HBM read) in
transposed [j, i] layout; a host boundary fix keeps the r<1 cutoff
bit-exact across the fp16 rounding. z is shipped transposed+fp16.

Per-core dataflow, 16 chunks of [128, 4096] (4 j-tiles):
  msg   = z @ (W*sqrt(pi)/2)  [N, D] fp16 via 16 PSUM batches (bias
          matmuls only if B is nonzero; B is zero in this problem).
          The sqrt(pi)/2 pre-scale host-side cancels Derivative_Erf's
          2/sqrt(pi) factor.
  m     = (r >= 1) * 60000          DVE ts chain (4x mode)
  t     = 1/r:  ACT raw Reciprocal for chunks in ACT_RECIP, else a
          DVE stock-op Newton chain (bitwise-NOT seed on the fp16 bit
          pattern + 1 Newton step; ~2.6e-3 rel err, plenty for the
          2e-2 gate) to offload the saturated ACT engine.
  t'    = max(t, m)                 DVE tt (masked elems -> 60000)
  w     = Derivative_Erf(sqrt2*t' - sqrt2) = (2/sqrt(pi))*exp(-2(t'-1)^2)
          ACT, immediate scale/bias; masked input saturates to exact 0.
  outT[d, i] += msg_chunk^T @ w     PE, PSUM fp32, 4 banks

ACT table sets are batched per 4-chunk super (recips then previous
super's D_Erfs) to bound table reloads. Reciprocal/Derivative_Erf are
emitted as raw InstActivation (wrapper vetoes Reciprocal on accuracy
grounds; measured ~1e-5 rel here, tolerance 2e-2).

This container's walrus encodes at most ONE semaphore wait per TPB
instruction; a post-Tile pass splits extra waits onto same-engine
EventSemaphore carriers (semantically identical, program order).
"""

import sys
import types

if "/opt/trn_rl_repo" not in sys.path:
    sys.path.insert(0, "/opt/trn_rl_repo")

import numpy as np

N = 8192
D = 256
NCORES = 8
ROWS = N // NCORES  # 1024 rows of dist per core
JT = 128  # j-tile (partition dim)
NJT = N // JT  # 64 j-tiles
CHUNK_JT = 4  # j-tiles per elementwise chunk
CHUNK_F = CHUNK_JT * ROWS  # free-dim elements per chunk tile (4096)
NCHUNK = NJT // CHUNK_JT  # 16
K = 4  # chunks per superchunk (ACT table-set batch)
NSUPER = NCHUNK // K  # 4

# chunks whose reciprocal runs on DVE (Newton) instead of ACT; spread so
# every super keeps both engines fed (concentrating them starves ACT)
DVE_RECIP = frozenset({3, 7, 11, 13, 15})
# msg PSUM->SBUF copy engine per batch: True -> ACT, False -> DVE
MSG_ON_ACT = tuple(False for _ in range(NCHUNK))

SQ2 = 1.4142135623730951
WSCALE = 0.8862269254527580  # sqrt(pi)/2, cancels D_Erf's 2/sqrt(pi)
# fp16 bitwise-NOT reciprocal seed + 1 Newton: y1 = c0*s*(c1 - r*c0*s),
# s = bitcast16(~bits16(r)). Constants minimax-fit over [0.05, 2.05].
NR_C0 = -0.23563272
NR_C1 = 2.00172757

_CACHE = {}


def _install_ntff_hook():
    """Provide antenv.axon_hooks (absent in this image) so trace=True can
    NTFF-profile through libaxon. Only needed for profiling runs."""
    if "antenv.axon_hooks" in sys.modules:
        return
    import antenv

    mod = types.ModuleType("antenv.axon_hooks")
    state = {"hook": None}
    mod.set_axon_ntff_profile_hook = lambda h: state.__setitem__("hook", h)
    mod.get_axon_ntff_profile_hook = lambda: state["hook"]
    sys.modules["antenv.axon_hooks"] = mod
    antenv.axon_hooks = mod
    try:
        from trn_agent_boot.trn_boot import _ntff_profile_via_ctypes

        mod.set_axon_ntff_profile_hook(
            _ntff_profile_via_ctypes("/opt/axon/libaxon_pjrt.so")
        )
    except Exception:
        pass


def _split_excess_waits(nc, max_waits=1):
    """Walrus here encodes at most one sync-wait per TPB instruction.
    Hoist extras onto preceding same-engine wait-only carriers."""
    import bass_rust

    seq = 0
    for fn in nc.m.functions:
        for bb in fn.blocks:
            insts = list(bb.instructions)
            out = []
            dirty = False
            for inst in insts:
                si = inst.sync_info
                if si is None:
                    out.append(inst)
                    continue
                waits = list(si.on_wait)
                if len(waits) > max_waits:
                    for w in waits[:-max_waits]:
                        seq += 1
                        carrier = bass_rust.InstEventSemaphore(
                            name=f"WSPLIT-{seq}", ins=[], outs=[]
                        )
                        carrier.engine = inst.engine
                        carrier.sync_info = bass_rust.SyncInfo(
                            on_wait=[w], on_update=[]
                        )
                        out.append(carrier)
                    inst.sync_info = bass_rust.SyncInfo(
                        on_wait=waits[-max_waits:], on_update=list(si.on_update)
                    )
                    dirty = True
                out.append(inst)
            if dirty:
                bb.instructions = out
    return seq


def _build(has_bias):
    import concourse.bass as bass
    import concourse.tile as tile
    from concourse import mybir

    f32 = mybir.dt.float32
    f16 = mybir.dt.float16
    u16 = mybir.dt.uint16
    AF = mybir.ActivationFunctionType
    OP = mybir.AluOpType

    nc = bass.Bass(
        "TRN2", target_bir_lowering=False, debug=False, num_devices=NCORES
    )
    distT_d = nc.dram_tensor("distT", [N, ROWS], f16, kind="ExternalInput").ap()
    zT_d = nc.dram_tensor("zT", [D, N], f16, kind="ExternalInput").ap()
    w_d = nc.dram_tensor("w", [D, D], f16, kind="ExternalInput").ap()
    b_d = nc.dram_tensor("b", [1, D], f16, kind="ExternalInput").ap()
    outT_d = nc.dram_tensor("outT", [D, ROWS], f32, kind="ExternalOutput").ap()

    def act_raw(out_ap, in_ap, func, bias, scale):
        return nc.scalar.add_instruction(
            mybir.InstActivation(
                name=nc.get_next_instruction_name(),
                func=func,
                ins=[
                    nc.scalar.lower_ap(in_ap),
                    mybir.ImmediateValue(dtype=f32, value=bias),
                    mybir.ImmediateValue(dtype=f32, value=scale),
                    mybir.ImmediateValue(dtype=f32, value=0.0),
                ],
                outs=[nc.scalar.lower_ap(out_ap)],
            )
        )

    with tile.TileContext(nc) as tc:
        with (
            tc.tile_pool(name="wb", bufs=1) as wbpool,
            tc.tile_pool(name="zq", bufs=1) as zqpool,
            tc.tile_pool(name="msgp", bufs=1) as msgpool,
            tc.tile_pool(name="msgps", bufs=2, space="PSUM") as msgpsum,
            tc.tile_pool(name="rch", bufs=4) as rpool,
            tc.tile_pool(name="mch", bufs=4) as mpool,
            tc.tile_pool(name="tch", bufs=5) as tpool,
            tc.tile_pool(name="nsc", bufs=1) as nscpool,
            tc.tile_pool(name="wch", bufs=2) as wpool,
            tc.tile_pool(name="outps", bufs=1, space="PSUM") as outpsum,
            tc.tile_pool(name="outsb", bufs=1) as outpool,
        ):
            # ---- W (fp16, host-scaled), optional bias, zT halves ----
            w_sb = [
                wbpool.tile([JT, D], f16, tag=f"wsb{h}", name=f"wsb{h}")
                for h in (0, 1)
            ]
            zq = [
                zqpool.tile([JT, N], f16, tag=f"zq{h}", name=f"zq{h}")
                for h in (0, 1)
            ]

            def load_z_quarter(q):
                for h in (0, 1):
                    nc.sync.dma_start(
                        zq[h][:, q * 2048 : (q + 1) * 2048],
                        zT_d[h * JT : (h + 1) * JT, q * 2048 : (q + 1) * 2048],
                    )

            def load_wb():
                for h in (0, 1):
                    nc.sync.dma_start(w_sb[h][:], w_d[h * JT : (h + 1) * JT, :])
                if has_bias:
                    nc.sync.dma_start(b_sb[:], b_d[:])
                    nc.gpsimd.memset(ones[:], 1.0)

            if has_bias:
                b_sb = wbpool.tile([1, D], f16)
                ones = wbpool.tile([1, JT], f16)

            msg = [
                msgpool.tile([JT, CHUNK_JT * D], f16, name=f"msg{b}", tag=f"msg{b}")
                for b in range(NCHUNK)
            ]
            acc = [
                outpsum.tile([JT, ROWS], f32, tag=f"acc{h}", name=f"acc{h}")
                for h in (0, 1)
            ]

            def emit_msg_matmuls(B):
                ps = msgpsum.tile([JT, CHUNK_JT * D], f32, name=f"mps{B}", tag="mps")
                for jj in range(CHUNK_JT):
                    jg = B * CHUNK_JT + jj
                    pslice = ps[:, jj * D : (jj + 1) * D]
                    for h in (0, 1):
                        nc.tensor.matmul(
                            pslice,
                            zq[h][:, jg * JT : (jg + 1) * JT],
                            w_sb[h][:],
                            start=(h == 0),
                            stop=(h == 1 and not has_bias),
                        )
                    if has_bias:
                        nc.tensor.matmul(
                            pslice, ones[:], b_sb[:], start=False, stop=True
                        )
                return ps

            def emit_msg_cast(B, ps):
                if MSG_ON_ACT[B]:
                    nc.scalar.copy(msg[B][:], ps[:])
                else:
                    nc.vector.tensor_copy(msg[B][:], ps[:])

            def emit_r_dma(c):
                r = rpool.tile([JT, CHUNK_F], f16, name=f"r{c}", tag="r")
                for k in range(CHUNK_JT):
                    jt = c * CHUNK_JT + k
                    nc.sync.dma_start(
                        r[:, k * ROWS : (k + 1) * ROWS],
                        distT_d[jt * JT : (jt + 1) * JT, :],
                    )
                return r

            def emit_p1_dve(c, r):
                """DVE-side phase 1: mask make, plus the Newton reciprocal
                for DVE_RECIP chunks."""
                m = mpool.tile([JT, CHUNK_F], f16, name=f"m{c}", tag="m")
                nc.vector.tensor_scalar(
                    m[:], r[:], 1.0, 60000.0, op0=OP.is_ge, op1=OP.mult
                )
                if c in DVE_RECIP:
                    # s = bitcast(~r); p = r*s (in place over the dead r);
                    # q = (p - c1/c0)*(-c0^2); t = s*q == c0*s*(c1 - r*c0*s)
                    t = tpool.tile([JT, CHUNK_F], f16, name=f"t{c}", tag="t")
                    s_t = nscpool.tile([JT, CHUNK_F], u16, name=f"ns{c}", tag="ns")
                    nc.vector.tensor_scalar(
                        s_t[:], r[:].bitcast(u16), 0, None, op0=OP.bitwise_not
                    )
                    nc.vector.tensor_tensor(
                        r[:], r[:], s_t[:].bitcast(f16), op=OP.mult
                    )
                    nc.vector.tensor_scalar(
                        r[:], r[:], NR_C1 / NR_C0, -NR_C0 * NR_C0,
                        op0=OP.subtract, op1=OP.mult,
                    )
                    nc.vector.tensor_tensor(
                        t[:], s_t[:].bitcast(f16), r[:], op=OP.mult
                    )
                    return c, None, t, m
                return c, r, None, m

            def emit_p1_act(entry):
                """ACT-side phase 1: the raw Reciprocal for ACT chunks."""
                c, r, t, m = entry
                if t is None:
                    t = tpool.tile([JT, CHUNK_F], f16, name=f"t{c}", tag="t")
                    act_raw(t[:], r[:], AF.Reciprocal, 0.0, 1.0)
                return c, t, m

            def emit_apply(c, t, m):
                # masked elements -> 60000; D_Erf saturates them to exact 0
                nc.vector.tensor_tensor(t[:], t[:], m[:], op=OP.max)

            def emit_derf_pe(c, t):
                w = wpool.tile([JT, CHUNK_F], f16, name=f"w{c}", tag="w")
                act_raw(w[:], t[:], AF.Derivative_Erf, -SQ2, SQ2)
                for k in range(CHUNK_JT):
                    jt = c * CHUNK_JT + k
                    mtile = msg[c]
                    for h in (0, 1):
                        lhsT = mtile[:, k * D + h * JT : k * D + (h + 1) * JT]
                        for nh in (0, 1):
                            nc.tensor.matmul(
                                acc[h][:, nh * 512 : (nh + 1) * 512],
                                lhsT,
                                w[:, k * ROWS + nh * 512 : k * ROWS + (nh + 1) * 512],
                                start=(jt == 0),
                                stop=(jt == NJT - 1),
                            )

            # ---- interleaved emission ----
            # Per-engine program orders (in-order engines!):
            #   DVE: apply(c), cast(c'), make(c'), newton(c') interleaved
            #        per chunk, so applies never queue behind a whole
            #        super's phase-1 burst.
            #   ACT: [derf run (D table)] then [recip run (R table)] per
            #        super: 2 table loads per super, and derfs aren't
            #        blocked behind recips that wait on fresh DMA.
            rr = [emit_r_dma(c) for c in range(K)]
            load_wb()
            load_z_quarter(0)
            load_z_quarter(1)
            part = [emit_p1_dve(c, rr[c]) for c in range(K)]
            pending = [emit_p1_act(e) for e in part]
            for B in range(K):
                emit_msg_cast(B, emit_msg_matmuls(B))
            for s in range(NSUPER):
                if s + 1 < NSUPER:
                    rr = [emit_r_dma(K * (s + 1) + k) for k in range(K)]
                if s + 2 < NSUPER:
                    load_z_quarter(s + 2)
                applied = []
                nxt_part = []
                pss = []
                for k in range(K):
                    c, t, m = pending[k]
                    emit_apply(c, t, m)
                    applied.append((c, t))
                    if s + 1 < NSUPER:
                        cn = K * (s + 1) + k
                        pss.append((cn, emit_msg_matmuls(cn)))
                        nxt_part.append(emit_p1_dve(cn, rr[k]))
                for cn, ps in pss:
                    emit_msg_cast(cn, ps)
                for c, t in applied:
                    emit_derf_pe(c, t)
                if s + 1 < NSUPER:
                    pending = [emit_p1_act(e) for e in nxt_part]

            # ---- tail: PSUM -> SBUF fp32 -> HBM ----
            for h in (0, 1):
                o = outpool.tile([JT, ROWS], f32, tag=f"o{h}", name=f"o{h}")
                nc.vector.tensor_copy(o[:], acc[h][:])
                nc.sync.dma_start(outT_d[h * JT : (h + 1) * JT, :], o[:])

    _split_excess_waits(nc)
    return nc


def kernel(z, dist_matrix, W, B, _trace=False):
    from concourse.bass_utils import run_bass_kernel_spmd

    if _trace:
        _install_ntff_hook()

    dist = np.asarray(dist_matrix, np.float32)
    z = np.asarray(z, np.float32)
    W_np = np.asarray(W, np.float32)
    B_np = np.asarray(B, np.float32).reshape(1, D)
    has_bias = bool(np.any(B_np))

    key = ("nc", has_bias)
    if key not in _CACHE:
        _CACHE[key] = _build(has_bias)
    nc = _CACHE[key]

    # fp16 dist with an exact cutoff: values < 1 that round UP to 1.0
    # would flip the mask; pin them to the largest fp16 below 1.
    r16 = dist.astype(np.float16)
    bad = (dist < 1.0) & (r16 >= 1.0)
    if bad.any():
        r16[bad] = np.float16(0.99951171875)

    zT16 = np.ascontiguousarray(z.T.astype(np.float16))
    W16 = (W_np * WSCALE).astype(np.float16)
    B16 = (B_np * WSCALE).astype(np.float16)

    in_maps = []
    for c in range(NCORES):
        blk = np.ascontiguousarray(r16[c * ROWS : (c + 1) * ROWS, :].T)
        in_maps.append({"distT": blk, "zT": zT16, "w": W16, "b": B16})

    res = run_bass_kernel_spmd(
        nc, in_maps, core_ids=list(range(NCORES)), trace=_trace
    )
    _CACHE["last"] = res

    out = np.empty((N, D), np.float32)
    for c in range(NCORES):
        out[c * ROWS : (c + 1) * ROWS, :] = res.results[c]["outT"].T
    return out


# revision 21
# speedup vs baseline: 1.1900x; 1.0080x over previous
"""nn_InteractionLayer Bass/Tile kernel for 8 Trainium2 NeuronCores.

out = where(dist < 1, exp(-2*(1/dist - 1)^2), 0) @ (z @ W + B)
N = 8192, D = 256.

Row-parallel: core c owns rows [c*1024, (c+1)*1024) of dist_matrix.
dist is shipped host-side as fp16 (halves the dominant HBM read) in
transposed [j, i] layout; a host boundary fix keeps the r<1 cutoff
bit-exact across the fp16 rounding. z is shipped transposed+fp16.

Per-core dataflow, 16 chunks of [128, 4096] (4 j-tiles):
  msg   = z @ (W*sqrt(pi)/2)  [N, D] fp16 via 16 PSUM batches (bias
          matmuls only if B is nonzero; B is zero in this problem).
          The sqrt(pi)/2 pre-scale host-side cancels Derivative_Erf's
          2/sqrt(pi) factor.
  m     = (r >= 1) * 60000          DVE ts chain (4x mode)
  t     = 1/r:  ACT raw Reciprocal for chunks in ACT_RECIP, else a
          DVE stock-op Newton chain (bitwise-NOT seed on the fp16 bit
          pattern + 1 Newton step; ~2.6e-3 rel err, plenty for the
          2e-2 gate) to offload the saturated ACT engine.
  t'    = max(t, m)                 DVE tt (masked elems -> 60000)
  w     = Derivative_Erf(sqrt2*t' - sqrt2) = (2/sqrt(pi))*exp(-2(t'-1)^2)
          ACT, immediate scale/bias; masked input saturates to exact 0.
  outT[d, i] += msg_chunk^T @ w     PE, PSUM fp32, 4 banks

ACT table sets are batched per 4-chunk super (recips then previous
super's D_Erfs) to bound table reloads. Reciprocal/Derivative_Erf are
emitted as raw InstActivation (wrapper vetoes Reciprocal on accuracy
grounds; measured ~1e-5 rel here, tolerance 2e-2).

This container's walrus encodes at most ONE semaphore wait per TPB
instruction; a post-Tile pass splits extra waits onto same-engine
EventSemaphore carriers (semantically identical, program order).
"""

import sys
import types

if "/opt/trn_rl_repo" not in sys.path:
    sys.path.insert(0, "/opt/trn_rl_repo")

import numpy as np

N = 8192
D = 256
NCORES = 8
ROWS = N // NCORES  # 1024 rows of dist per core
JT = 128  # j-tile (partition dim)
NJT = N // JT  # 64 j-tiles
CHUNK_JT = 4  # j-tiles per elementwise chunk
CHUNK_F = CHUNK_JT * ROWS  # free-dim elements per chunk tile (4096)
NCHUNK = NJT // CHUNK_JT  # 16
K = 4  # chunks per superchunk (ACT table-set batch)
NSUPER = NCHUNK // K  # 4

# chunks whose reciprocal runs on DVE (Newton) instead of ACT; spread so
# every super keeps both engines fed (concentrating them starves ACT)
DVE_RECIP = frozenset({3, 7, 11, 13, 15})
# msg PSUM->SBUF copy engine per batch: True -> ACT, False -> DVE
MSG_ON_ACT = tuple(False for _ in range(NCHUNK))

SQ2 = 1.4142135623730951
WSCALE = 0.8862269254527580  # sqrt(pi)/2, cancels D_Erf's 2/sqrt(pi)
# fp16 bitwise-NOT reciprocal seed + 1 Newton: y1 = c0*s*(c1 - r*c0*s),
# s = bitcast16(~bits16(r)). Constants minimax-fit over [0.05, 2.05].
NR_C0 = -0.23563272
NR_C1 = 2.00172757

_CACHE = {}


def _install_ntff_hook():
    """Provide antenv.axon_hooks (absent in this image) so trace=True can
    NTFF-profile through libaxon. Only needed for profiling runs."""
    if "antenv.axon_hooks" in sys.modules:
        return
    import antenv

    mod = types.ModuleType("antenv.axon_hooks")
    state = {"hook": None}
    mod.set_axon_ntff_profile_hook = lambda h: state.__setitem__("hook", h)
    mod.get_axon_ntff_profile_hook = lambda: state["hook"]
    sys.modules["antenv.axon_hooks"] = mod
    antenv.axon_hooks = mod
    try:
        from trn_agent_boot.trn_boot import _ntff_profile_via_ctypes

        mod.set_axon_ntff_profile_hook(
            _ntff_profile_via_ctypes("/opt/axon/libaxon_pjrt.so")
        )
    except Exception:
        pass


def _split_excess_waits(nc, max_waits=1):
    """Walrus here encodes at most one sync-wait per TPB instruction.
    Hoist extras onto preceding same-engine wait-only carriers."""
    import bass_rust

    seq = 0
    for fn in nc.m.functions:
        for bb in fn.blocks:
            insts = list(bb.instructions)
            out = []
            dirty = False
            for inst in insts:
                si = inst.sync_info
                if si is None:
                    out.append(inst)
                    continue
                waits = list(si.on_wait)
                if len(waits) > max_waits:
                    for w in waits[:-max_waits]:
                        seq += 1
                        carrier = bass_rust.InstEventSemaphore(
                            name=f"WSPLIT-{seq}", ins=[], outs=[]
                        )
                        carrier.engine = inst.engine
                        carrier.sync_info = bass_rust.SyncInfo(
                            on_wait=[w], on_update=[]
                        )
                        out.append(carrier)
                    inst.sync_info = bass_rust.SyncInfo(
                        on_wait=waits[-max_waits:], on_update=list(si.on_update)
                    )
                    dirty = True
                out.append(inst)
            if dirty:
                bb.instructions = out
    return seq


def _build(has_bias):
    import concourse.bass as bass
    import concourse.tile as tile
    from concourse import mybir

    f32 = mybir.dt.float32
    f16 = mybir.dt.float16
    u16 = mybir.dt.uint16
    AF = mybir.ActivationFunctionType
    OP = mybir.AluOpType

    nc = bass.Bass(
        "TRN2", target_bir_lowering=False, debug=False, num_devices=NCORES
    )
    distT_d = nc.dram_tensor("distT", [N, ROWS], f16, kind="ExternalInput").ap()
    zT_d = nc.dram_tensor("zT", [D, N], f16, kind="ExternalInput").ap()
    w_d = nc.dram_tensor("w", [D, D], f16, kind="ExternalInput").ap()
    b_d = nc.dram_tensor("b", [1, D], f16, kind="ExternalInput").ap()
    outT_d = nc.dram_tensor("outT", [D, ROWS], f32, kind="ExternalOutput").ap()

    def act_raw(out_ap, in_ap, func, bias, scale):
        return nc.scalar.add_instruction(
            mybir.InstActivation(
                name=nc.get_next_instruction_name(),
                func=func,
                ins=[
                    nc.scalar.lower_ap(in_ap),
                    mybir.ImmediateValue(dtype=f32, value=bias),
                    mybir.ImmediateValue(dtype=f32, value=scale),
                    mybir.ImmediateValue(dtype=f32, value=0.0),
                ],
                outs=[nc.scalar.lower_ap(out_ap)],
            )
        )

    with tile.TileContext(nc) as tc:
        with (
            tc.tile_pool(name="wb", bufs=1) as wbpool,
            tc.tile_pool(name="zq", bufs=1) as zqpool,
            tc.tile_pool(name="msgp", bufs=1) as msgpool,
            tc.tile_pool(name="msgps", bufs=2, space="PSUM") as msgpsum,
            tc.tile_pool(name="rch", bufs=4) as rpool,
            tc.tile_pool(name="mch", bufs=4) as mpool,
            tc.tile_pool(name="tch", bufs=5) as tpool,
            tc.tile_pool(name="nsc", bufs=1) as nscpool,
            tc.tile_pool(name="wch", bufs=2) as wpool,
            tc.tile_pool(name="outps", bufs=1, space="PSUM") as outpsum,
            tc.tile_pool(name="outsb", bufs=1) as outpool,
        ):
            # ---- W (fp16, host-scaled), optional bias, zT halves ----
            w_sb = [
                wbpool.tile([JT, D], f16, tag=f"wsb{h}", name=f"wsb{h}")
                for h in (0, 1)
            ]
            zq = [
                zqpool.tile([JT, N], f16, tag=f"zq{h}", name=f"zq{h}")
                for h in (0, 1)
            ]

            def load_z_quarter(q):
                for h in (0, 1):
                    nc.sync.dma_start(
                        zq[h][:, q * 2048 : (q + 1) * 2048],
                        zT_d[h * JT : (h + 1) * JT, q * 2048 : (q + 1) * 2048],
                    )

            def load_wb():
                for h in (0, 1):
                    nc.sync.dma_start(w_sb[h][:], w_d[h * JT : (h + 1) * JT, :])
                if has_bias:
                    nc.sync.dma_start(b_sb[:], b_d[:])
                    nc.gpsimd.memset(ones[:], 1.0)

            if has_bias:
                b_sb = wbpool.tile([1, D], f16)
                ones = wbpool.tile([1, JT], f16)

            msg = [
                msgpool.tile([JT, CHUNK_JT * D], f16, name=f"msg{b}", tag=f"msg{b}")
                for b in range(NCHUNK)
            ]
            acc = [
                outpsum.tile([JT, ROWS], f32, tag=f"acc{h}", name=f"acc{h}")
                for h in (0, 1)
            ]

            def emit_msg_matmuls(B):
                ps = msgpsum.tile([JT, CHUNK_JT * D], f32, name=f"mps{B}", tag="mps")
                for jj in range(CHUNK_JT):
                    jg = B * CHUNK_JT + jj
                    pslice = ps[:, jj * D : (jj + 1) * D]
                    for h in (0, 1):
                        nc.tensor.matmul(
                            pslice,
                            zq[h][:, jg * JT : (jg + 1) * JT],
                            w_sb[h][:],
                            start=(h == 0),
                            stop=(h == 1 and not has_bias),
                        )
                    if has_bias:
                        nc.tensor.matmul(
                            pslice, ones[:], b_sb[:], start=False, stop=True
                        )
                return ps

            def emit_msg_cast(B, ps):
                if MSG_ON_ACT[B]:
                    nc.scalar.copy(msg[B][:], ps[:])
                else:
                    nc.vector.tensor_copy(msg[B][:], ps[:])

            def emit_r_dma(c):
                r = rpool.tile([JT, CHUNK_F], f16, name=f"r{c}", tag="r")
                for k in range(CHUNK_JT):
                    jt = c * CHUNK_JT + k
                    nc.sync.dma_start(
                        r[:, k * ROWS : (k + 1) * ROWS],
                        distT_d[jt * JT : (jt + 1) * JT, :],
                    )
                return r

            def emit_p1_dve(c, r):
                """DVE-side phase 1: mask make, plus the Newton reciprocal
                for DVE_RECIP chunks."""
                m = mpool.tile([JT, CHUNK_F], f16, name=f"m{c}", tag="m")
                nc.vector.tensor_scalar(
                    m[:], r[:], 1.0, 60000.0, op0=OP.is_ge, op1=OP.mult
                )
                if c in DVE_RECIP:
                    # s = bitcast(~r); p = r*s (in place over the dead r);
                    # q = (p - c1/c0)*(-c0^2); t = s*q == c0*s*(c1 - r*c0*s)
                    t = tpool.tile([JT, CHUNK_F], f16, name=f"t{c}", tag="t")
                    s_t = nscpool.tile([JT, CHUNK_F], u16, name=f"ns{c}", tag="ns")
                    nc.vector.tensor_scalar(
                        s_t[:], r[:].bitcast(u16), 0, None, op0=OP.bitwise_not
                    )
                    nc.vector.tensor_tensor(
                        r[:], r[:], s_t[:].bitcast(f16), op=OP.mult
                    )
                    nc.vector.tensor_scalar(
                        r[:], r[:], NR_C1 / NR_C0, -NR_C0 * NR_C0,
                        op0=OP.subtract, op1=OP.mult,
                    )
                    nc.vector.tensor_tensor(
                        t[:], s_t[:].bitcast(f16), r[:], op=OP.mult
                    )
                    return c, None, t, m
                return c, r, None, m

            def emit_p1_act(entry):
                """ACT-side phase 1: the raw Reciprocal for ACT chunks."""
                c, r, t, m = entry
                if t is None:
                    t = tpool.tile([JT, CHUNK_F], f16, name=f"t{c}", tag="t")
                    act_raw(t[:], r[:], AF.Reciprocal, 0.0, 1.0)
                return c, t, m

            def emit_apply(c, t, m):
                # masked elements -> 60000; D_Erf saturates them to exact 0
                nc.vector.tensor_tensor(t[:], t[:], m[:], op=OP.max)

            def emit_derf_pe(c, t):
                w = wpool.tile([JT, CHUNK_F], f16, name=f"w{c}", tag="w")
                act_raw(w[:], t[:], AF.Derivative_Erf, -SQ2, SQ2)
                for k in range(CHUNK_JT):
                    jt = c * CHUNK_JT + k
                    mtile = msg[c]
                    for h in (0, 1):
                        lhsT = mtile[:, k * D + h * JT : k * D + (h + 1) * JT]
                        for nh in (0, 1):
                            nc.tensor.matmul(
                                acc[h][:, nh * 512 : (nh + 1) * 512],
                                lhsT,
                                w[:, k * ROWS + nh * 512 : k * ROWS + (nh + 1) * 512],
                                start=(jt == 0),
                                stop=(jt == NJT - 1),
                            )

            # ---- interleaved emission ----
            # Per-engine program orders (in-order engines!):
            #   DVE: apply(c), cast(c'), make(c'), newton(c') interleaved
            #        per chunk, so applies never queue behind a whole
            #        super's phase-1 burst.
            #   ACT: [derf run (D table)] then [recip run (R table)] per
            #        super: 2 table loads per super, and derfs aren't
            #        blocked behind recips that wait on fresh DMA.
            load_wb()
            load_z_quarter(0)
            rr = [emit_r_dma(c) for c in range(K)]
            load_z_quarter(1)
            part = [emit_p1_dve(c, rr[c]) for c in range(K)]
            pending = [emit_p1_act(e) for e in part]
            for B in range(K):
                emit_msg_cast(B, emit_msg_matmuls(B))
            for s in range(NSUPER):
                if s + 1 < NSUPER:
                    rr = [emit_r_dma(K * (s + 1) + k) for k in range(K)]
                if s + 2 < NSUPER:
                    load_z_quarter(s + 2)
                applied = []
                nxt_part = []
                pss = []
                for k in range(K):
                    c, t, m = pending[k]
                    emit_apply(c, t, m)
                    applied.append((c, t))
                    if s + 1 < NSUPER:
                        cn = K * (s + 1) + k
                        pss.append((cn, emit_msg_matmuls(cn)))
                        nxt_part.append(emit_p1_dve(cn, rr[k]))
                for cn, ps in pss:
                    emit_msg_cast(cn, ps)
                # recips for super s+1 go to ACT *before* this super's
                # derfs, so the next super's applies are never starved;
                # derfs can afford the delay (PE consumes them with slack).
                if s + 1 < NSUPER:
                    pending = [emit_p1_act(e) for e in nxt_part]
                for c, t in applied:
                    emit_derf_pe(c, t)

            # ---- tail: PSUM -> SBUF fp32 -> HBM ----
            for h in (0, 1):
                o = outpool.tile([JT, ROWS], f32, tag=f"o{h}", name=f"o{h}")
                nc.vector.tensor_copy(o[:], acc[h][:])
                nc.sync.dma_start(outT_d[h * JT : (h + 1) * JT, :], o[:])

    _split_excess_waits(nc)
    return nc


def kernel(z, dist_matrix, W, B, _trace=False):
    from concourse.bass_utils import run_bass_kernel_spmd

    if _trace:
        _install_ntff_hook()

    dist = np.asarray(dist_matrix, np.float32)
    z = np.asarray(z, np.float32)
    W_np = np.asarray(W, np.float32)
    B_np = np.asarray(B, np.float32).reshape(1, D)
    has_bias = bool(np.any(B_np))

    key = ("nc", has_bias)
    if key not in _CACHE:
        _CACHE[key] = _build(has_bias)
    nc = _CACHE[key]

    # fp16 dist with an exact cutoff: values < 1 that round UP to 1.0
    # would flip the mask; pin them to the largest fp16 below 1.
    r16 = dist.astype(np.float16)
    bad = (dist < 1.0) & (r16 >= 1.0)
    if bad.any():
        r16[bad] = np.float16(0.99951171875)

    zT16 = np.ascontiguousarray(z.T.astype(np.float16))
    W16 = (W_np * WSCALE).astype(np.float16)
    B16 = (B_np * WSCALE).astype(np.float16)

    in_maps = []
    for c in range(NCORES):
        blk = np.ascontiguousarray(r16[c * ROWS : (c + 1) * ROWS, :].T)
        in_maps.append({"distT": blk, "zT": zT16, "w": W16, "b": B16})

    res = run_bass_kernel_spmd(
        nc, in_maps, core_ids=list(range(NCORES)), trace=_trace
    )
    _CACHE["last"] = res

    out = np.empty((N, D), np.float32)
    for c in range(NCORES):
        out[c * ROWS : (c + 1) * ROWS, :] = res.results[c]["outT"].T
    return out


# revision 23
# speedup vs baseline: 1.1951x; 1.0043x over previous
"""nn_InteractionLayer Bass/Tile kernel for 8 Trainium2 NeuronCores.

out = where(dist < 1, exp(-2*(1/dist - 1)^2), 0) @ (z @ W + B)
N = 8192, D = 256.

Row-parallel: core c owns rows [c*1024, (c+1)*1024) of dist_matrix.
dist is shipped host-side as fp16 (halves the dominant HBM read) in
transposed [j, i] layout; a host boundary fix keeps the r<1 cutoff
bit-exact across the fp16 rounding. z is shipped transposed+fp16.

Per-core dataflow, 16 chunks of [128, 4096] (4 j-tiles):
  msg   = z @ (W*sqrt(pi)/2)  [N, D] fp16 via 16 PSUM batches (bias
          matmuls only if B is nonzero; B is zero in this problem).
          The sqrt(pi)/2 pre-scale host-side cancels Derivative_Erf's
          2/sqrt(pi) factor.
  m     = (r >= 1) * 60000          DVE ts chain (4x mode)
  t     = 1/r:  ACT raw Reciprocal for chunks in ACT_RECIP, else a
          DVE stock-op Newton chain (bitwise-NOT seed on the fp16 bit
          pattern + 1 Newton step; ~2.6e-3 rel err, plenty for the
          2e-2 gate) to offload the saturated ACT engine.
  t'    = max(t, m)                 DVE tt (masked elems -> 60000)
  w     = Derivative_Erf(sqrt2*t' - sqrt2) = (2/sqrt(pi))*exp(-2(t'-1)^2)
          ACT, immediate scale/bias; masked input saturates to exact 0.
  outT[d, i] += msg_chunk^T @ w     PE, PSUM fp32, 4 banks

ACT table sets are batched per 4-chunk super (recips then previous
super's D_Erfs) to bound table reloads. Reciprocal/Derivative_Erf are
emitted as raw InstActivation (wrapper vetoes Reciprocal on accuracy
grounds; measured ~1e-5 rel here, tolerance 2e-2).

This container's walrus encodes at most ONE semaphore wait per TPB
instruction; a post-Tile pass splits extra waits onto same-engine
EventSemaphore carriers (semantically identical, program order).
"""

import sys
import types

if "/opt/trn_rl_repo" not in sys.path:
    sys.path.insert(0, "/opt/trn_rl_repo")

import numpy as np

N = 8192
D = 256
NCORES = 8
ROWS = N // NCORES  # 1024 rows of dist per core
JT = 128  # j-tile (partition dim)
NJT = N // JT  # 64 j-tiles
CHUNK_JT = 4  # j-tiles per elementwise chunk
CHUNK_F = CHUNK_JT * ROWS  # free-dim elements per chunk tile (4096)
NCHUNK = NJT // CHUNK_JT  # 16
K = 4  # chunks per superchunk (ACT table-set batch)
NSUPER = NCHUNK // K  # 4

# chunks whose reciprocal runs on DVE (Newton) instead of ACT; spread so
# every super keeps both engines fed (concentrating them starves ACT)
DVE_RECIP = frozenset({3, 7, 11, 13, 15})
# msg PSUM->SBUF copy engine per batch: True -> ACT, False -> DVE
MSG_ON_ACT = tuple(False for _ in range(NCHUNK))

SQ2 = 1.4142135623730951
WSCALE = 0.8862269254527580  # sqrt(pi)/2, cancels D_Erf's 2/sqrt(pi)
# fp16 bitwise-NOT reciprocal seed + 1 Newton: y1 = c0*s*(c1 - r*c0*s),
# s = bitcast16(~bits16(r)). Constants minimax-fit over [0.05, 2.05].
NR_C0 = -0.23563272
NR_C1 = 2.00172757

_CACHE = {}


def _install_ntff_hook():
    """Provide antenv.axon_hooks (absent in this image) so trace=True can
    NTFF-profile through libaxon. Only needed for profiling runs."""
    if "antenv.axon_hooks" in sys.modules:
        return
    import antenv

    mod = types.ModuleType("antenv.axon_hooks")
    state = {"hook": None}
    mod.set_axon_ntff_profile_hook = lambda h: state.__setitem__("hook", h)
    mod.get_axon_ntff_profile_hook = lambda: state["hook"]
    sys.modules["antenv.axon_hooks"] = mod
    antenv.axon_hooks = mod
    try:
        from trn_agent_boot.trn_boot import _ntff_profile_via_ctypes

        mod.set_axon_ntff_profile_hook(
            _ntff_profile_via_ctypes("/opt/axon/libaxon_pjrt.so")
        )
    except Exception:
        pass


def _split_excess_waits(nc, max_waits=1):
    """Walrus here encodes at most one sync-wait per TPB instruction.
    Hoist extras onto preceding same-engine wait-only carriers."""
    import bass_rust

    seq = 0
    for fn in nc.m.functions:
        for bb in fn.blocks:
            insts = list(bb.instructions)
            out = []
            dirty = False
            for inst in insts:
                si = inst.sync_info
                if si is None:
                    out.append(inst)
                    continue
                waits = list(si.on_wait)
                if len(waits) > max_waits:
                    for w in waits[:-max_waits]:
                        seq += 1
                        carrier = bass_rust.InstEventSemaphore(
                            name=f"WSPLIT-{seq}", ins=[], outs=[]
                        )
                        carrier.engine = inst.engine
                        carrier.sync_info = bass_rust.SyncInfo(
                            on_wait=[w], on_update=[]
                        )
                        out.append(carrier)
                    inst.sync_info = bass_rust.SyncInfo(
                        on_wait=waits[-max_waits:], on_update=list(si.on_update)
                    )
                    dirty = True
                out.append(inst)
            if dirty:
                bb.instructions = out
    return seq


def _build(has_bias):
    import concourse.bass as bass
    import concourse.tile as tile
    from concourse import mybir

    f32 = mybir.dt.float32
    f16 = mybir.dt.float16
    u16 = mybir.dt.uint16
    AF = mybir.ActivationFunctionType
    OP = mybir.AluOpType

    nc = bass.Bass(
        "TRN2", target_bir_lowering=False, debug=False, num_devices=NCORES
    )
    distT_d = nc.dram_tensor("distT", [N, ROWS], f16, kind="ExternalInput").ap()
    zT_d = nc.dram_tensor("zT", [D, N], f16, kind="ExternalInput").ap()
    w_d = nc.dram_tensor("w", [D, D], f16, kind="ExternalInput").ap()
    b_d = nc.dram_tensor("b", [1, D], f16, kind="ExternalInput").ap()
    outT_d = nc.dram_tensor("outT", [D, ROWS], f32, kind="ExternalOutput").ap()

    def act_raw(out_ap, in_ap, func, bias, scale):
        return nc.scalar.add_instruction(
            mybir.InstActivation(
                name=nc.get_next_instruction_name(),
                func=func,
                ins=[
                    nc.scalar.lower_ap(in_ap),
                    mybir.ImmediateValue(dtype=f32, value=bias),
                    mybir.ImmediateValue(dtype=f32, value=scale),
                    mybir.ImmediateValue(dtype=f32, value=0.0),
                ],
                outs=[nc.scalar.lower_ap(out_ap)],
            )
        )

    with tile.TileContext(nc) as tc:
        with (
            tc.tile_pool(name="wb", bufs=1) as wbpool,
            tc.tile_pool(name="zq", bufs=1) as zqpool,
            tc.tile_pool(name="msgp", bufs=1) as msgpool,
            tc.tile_pool(name="msgps", bufs=2, space="PSUM") as msgpsum,
            tc.tile_pool(name="rch", bufs=4) as rpool,
            tc.tile_pool(name="mch", bufs=4) as mpool,
            tc.tile_pool(name="tch", bufs=5) as tpool,
            tc.tile_pool(name="nsc", bufs=1) as nscpool,
            tc.tile_pool(name="wch", bufs=2) as wpool,
            tc.tile_pool(name="outps", bufs=1, space="PSUM") as outpsum,
            tc.tile_pool(name="outsb", bufs=1) as outpool,
        ):
            # ---- W (fp16, host-scaled), optional bias, zT halves ----
            w_sb = [
                wbpool.tile([JT, D], f16, tag=f"wsb{h}", name=f"wsb{h}")
                for h in (0, 1)
            ]
            zq = [
                zqpool.tile([JT, N], f16, tag=f"zq{h}", name=f"zq{h}")
                for h in (0, 1)
            ]

            def load_z_quarter(q):
                for h in (0, 1):
                    nc.sync.dma_start(
                        zq[h][:, q * 2048 : (q + 1) * 2048],
                        zT_d[h * JT : (h + 1) * JT, q * 2048 : (q + 1) * 2048],
                    )

            def load_wb():
                for h in (0, 1):
                    nc.sync.dma_start(w_sb[h][:], w_d[h * JT : (h + 1) * JT, :])
                if has_bias:
                    nc.sync.dma_start(b_sb[:], b_d[:])
                    nc.gpsimd.memset(ones[:], 1.0)

            if has_bias:
                b_sb = wbpool.tile([1, D], f16)
                ones = wbpool.tile([1, JT], f16)

            msg = [
                msgpool.tile([JT, CHUNK_JT * D], f16, name=f"msg{b}", tag=f"msg{b}")
                for b in range(NCHUNK)
            ]
            acc = [
                outpsum.tile([JT, ROWS], f32, tag=f"acc{h}", name=f"acc{h}")
                for h in (0, 1)
            ]

            def emit_msg_matmuls(B):
                ps = msgpsum.tile([JT, CHUNK_JT * D], f32, name=f"mps{B}", tag="mps")
                for jj in range(CHUNK_JT):
                    jg = B * CHUNK_JT + jj
                    pslice = ps[:, jj * D : (jj + 1) * D]
                    for h in (0, 1):
                        nc.tensor.matmul(
                            pslice,
                            zq[h][:, jg * JT : (jg + 1) * JT],
                            w_sb[h][:],
                            start=(h == 0),
                            stop=(h == 1 and not has_bias),
                        )
                    if has_bias:
                        nc.tensor.matmul(
                            pslice, ones[:], b_sb[:], start=False, stop=True
                        )
                return ps

            def emit_msg_cast(B, ps):
                if MSG_ON_ACT[B]:
                    nc.scalar.copy(msg[B][:], ps[:])
                else:
                    nc.vector.tensor_copy(msg[B][:], ps[:])

            def emit_r_dma(c):
                r = rpool.tile([JT, CHUNK_F], f16, name=f"r{c}", tag="r")
                for k in range(CHUNK_JT):
                    jt = c * CHUNK_JT + k
                    nc.sync.dma_start(
                        r[:, k * ROWS : (k + 1) * ROWS],
                        distT_d[jt * JT : (jt + 1) * JT, :],
                    )
                return r

            def emit_p1_dve(c, r):
                """DVE-side phase 1: mask make, plus the Newton reciprocal
                for DVE_RECIP chunks."""
                m = mpool.tile([JT, CHUNK_F], f16, name=f"m{c}", tag="m")
                nc.vector.tensor_scalar(
                    m[:], r[:], 1.0, 60000.0, op0=OP.is_ge, op1=OP.mult
                )
                if c in DVE_RECIP:
                    # s = bitcast(~r); p = r*s (in place over the dead r);
                    # q = (p - c1/c0)*(-c0^2); t = s*q == c0*s*(c1 - r*c0*s)
                    t = tpool.tile([JT, CHUNK_F], f16, name=f"t{c}", tag="t")
                    s_t = nscpool.tile([JT, CHUNK_F], u16, name=f"ns{c}", tag="ns")
                    nc.vector.tensor_scalar(
                        s_t[:], r[:].bitcast(u16), 0, None, op0=OP.bitwise_not
                    )
                    nc.vector.tensor_tensor(
                        r[:], r[:], s_t[:].bitcast(f16), op=OP.mult
                    )
                    nc.vector.tensor_scalar(
                        r[:], r[:], NR_C1 / NR_C0, -NR_C0 * NR_C0,
                        op0=OP.subtract, op1=OP.mult,
                    )
                    nc.vector.tensor_tensor(
                        t[:], s_t[:].bitcast(f16), r[:], op=OP.mult
                    )
                    return c, None, t, m
                return c, r, None, m

            def emit_p1_act(entry):
                """ACT-side phase 1: the raw Reciprocal for ACT chunks."""
                c, r, t, m = entry
                if t is None:
                    t = tpool.tile([JT, CHUNK_F], f16, name=f"t{c}", tag="t")
                    act_raw(t[:], r[:], AF.Reciprocal, 0.0, 1.0)
                return c, t, m

            def emit_apply(c, t, m):
                # masked elements -> 60000; D_Erf saturates them to exact 0
                nc.vector.tensor_tensor(t[:], t[:], m[:], op=OP.max)

            def emit_derf_pe(c, t):
                w = wpool.tile([JT, CHUNK_F], f16, name=f"w{c}", tag="w")
                act_raw(w[:], t[:], AF.Derivative_Erf, -SQ2, SQ2)
                for k in range(CHUNK_JT):
                    jt = c * CHUNK_JT + k
                    mtile = msg[c]
                    for h in (0, 1):
                        lhsT = mtile[:, k * D + h * JT : k * D + (h + 1) * JT]
                        for nh in (0, 1):
                            nc.tensor.matmul(
                                acc[h][:, nh * 512 : (nh + 1) * 512],
                                lhsT,
                                w[:, k * ROWS + nh * 512 : k * ROWS + (nh + 1) * 512],
                                start=(jt == 0),
                                stop=(jt == NJT - 1),
                            )

            # ---- interleaved emission ----
            # Per-engine program orders (in-order engines!):
            #   DVE: apply(c), cast(c'), make(c'), newton(c') interleaved
            #        per chunk, so applies never queue behind a whole
            #        super's phase-1 burst.
            #   ACT: [derf run (D table)] then [recip run (R table)] per
            #        super: 2 table loads per super, and derfs aren't
            #        blocked behind recips that wait on fresh DMA.
            load_wb()
            load_z_quarter(0)
            rr = [emit_r_dma(c) for c in range(K)]
            part = [emit_p1_dve(c, rr[c]) for c in range(K)]
            pending = [emit_p1_act(e) for e in part]
            for B in range(K):
                emit_msg_cast(B, emit_msg_matmuls(B))
            for s in range(NSUPER):
                if s + 1 < NSUPER:
                    rr = [emit_r_dma(K * (s + 1) + k) for k in range(K)]
                    load_z_quarter(s + 1)
                applied = []
                nxt_part = []
                pss = []
                for k in range(K):
                    c, t, m = pending[k]
                    emit_apply(c, t, m)
                    applied.append((c, t))
                    if s + 1 < NSUPER:
                        cn = K * (s + 1) + k
                        pss.append((cn, emit_msg_matmuls(cn)))
                        nxt_part.append(emit_p1_dve(cn, rr[k]))
                for cn, ps in pss:
                    emit_msg_cast(cn, ps)
                # From super 1 on, recips for super s+1 go to ACT *before*
                # this super's derfs so the next super's applies are never
                # starved (derfs can afford the delay; PE has slack). In
                # super 0 the r DMAs are still in flight, so derfs first.
                if s == 0:
                    for c, t in applied:
                        emit_derf_pe(c, t)
                    pending = [emit_p1_act(e) for e in nxt_part]
                else:
                    if s + 1 < NSUPER:
                        pending = [emit_p1_act(e) for e in nxt_part]
                    for c, t in applied:
                        emit_derf_pe(c, t)

            # ---- tail: PSUM -> SBUF fp32 -> HBM ----
            for h in (0, 1):
                o = outpool.tile([JT, ROWS], f32, tag=f"o{h}", name=f"o{h}")
                nc.vector.tensor_copy(o[:], acc[h][:])
                nc.sync.dma_start(outT_d[h * JT : (h + 1) * JT, :], o[:])

    _split_excess_waits(nc)
    return nc


def kernel(z, dist_matrix, W, B, _trace=False):
    from concourse.bass_utils import run_bass_kernel_spmd

    if _trace:
        _install_ntff_hook()

    dist = np.asarray(dist_matrix, np.float32)
    z = np.asarray(z, np.float32)
    W_np = np.asarray(W, np.float32)
    B_np = np.asarray(B, np.float32).reshape(1, D)
    has_bias = bool(np.any(B_np))

    key = ("nc", has_bias)
    if key not in _CACHE:
        _CACHE[key] = _build(has_bias)
    nc = _CACHE[key]

    # fp16 dist with an exact cutoff: values < 1 that round UP to 1.0
    # would flip the mask; pin them to the largest fp16 below 1.
    r16 = dist.astype(np.float16)
    bad = (dist < 1.0) & (r16 >= 1.0)
    if bad.any():
        r16[bad] = np.float16(0.99951171875)

    zT16 = np.ascontiguousarray(z.T.astype(np.float16))
    W16 = (W_np * WSCALE).astype(np.float16)
    B16 = (B_np * WSCALE).astype(np.float16)

    in_maps = []
    for c in range(NCORES):
        blk = np.ascontiguousarray(r16[c * ROWS : (c + 1) * ROWS, :].T)
        in_maps.append({"distT": blk, "zT": zT16, "w": W16, "b": B16})

    res = run_bass_kernel_spmd(
        nc, in_maps, core_ids=list(range(NCORES)), trace=_trace
    )
    _CACHE["last"] = res

    out = np.empty((N, D), np.float32)
    for c in range(NCORES):
        out[c * ROWS : (c + 1) * ROWS, :] = res.results[c]["outT"].T
    return out


# revision 26
# speedup vs baseline: 1.2517x; 1.0473x over previous
"""nn_InteractionLayer Bass/Tile kernel for 8 Trainium2 NeuronCores.

out = where(dist < 1, exp(-2*(1/dist - 1)^2), 0) @ (z @ W + B)
N = 8192, D = 256.

Row-parallel: core c owns rows [c*1024, (c+1)*1024) of dist_matrix.
dist is shipped host-side as fp16 (halves the dominant HBM read) in
transposed [j, i] layout; a host boundary fix keeps the r<1 cutoff
bit-exact across the fp16 rounding. z is shipped transposed+fp16.

Per-core dataflow, 16 chunks of [128, 4096] (4 j-tiles):
  msg   = z @ (W*sqrt(pi)/2)  [N, D] fp16 via 16 PSUM batches (bias
          matmuls only if B is nonzero; B is zero in this problem).
          The sqrt(pi)/2 pre-scale host-side cancels Derivative_Erf's
          2/sqrt(pi) factor.
  m     = (r >= 1) * 60000          DVE ts chain (4x mode)
  t     = 1/r:  ACT raw Reciprocal for chunks in ACT_RECIP, else a
          DVE stock-op Newton chain (bitwise-NOT seed on the fp16 bit
          pattern + 1 Newton step; ~2.6e-3 rel err, plenty for the
          2e-2 gate) to offload the saturated ACT engine.
  t'    = max(t, m)                 DVE tt (masked elems -> 60000)
  w     = Derivative_Erf(sqrt2*t' - sqrt2) = (2/sqrt(pi))*exp(-2(t'-1)^2)
          ACT, immediate scale/bias; masked input saturates to exact 0.
  outT[d, i] += msg_chunk^T @ w     PE, PSUM fp32, 4 banks

ACT table sets are batched per 4-chunk super (recips then previous
super's D_Erfs) to bound table reloads. Reciprocal/Derivative_Erf are
emitted as raw InstActivation (wrapper vetoes Reciprocal on accuracy
grounds; measured ~1e-5 rel here, tolerance 2e-2).

This container's walrus encodes at most ONE semaphore wait per TPB
instruction; a post-Tile pass splits extra waits onto same-engine
EventSemaphore carriers (semantically identical, program order).
"""

import sys
import types

if "/opt/trn_rl_repo" not in sys.path:
    sys.path.insert(0, "/opt/trn_rl_repo")

import numpy as np

N = 8192
D = 256
NCORES = 8
ROWS = N // NCORES  # 1024 rows of dist per core
JT = 128  # j-tile (partition dim)
NJT = N // JT  # 64 j-tiles
CHUNK_JT = 4  # j-tiles per elementwise chunk
CHUNK_F = CHUNK_JT * ROWS  # free-dim elements per chunk tile (4096)
NCHUNK = NJT // CHUNK_JT  # 16
K = 4  # chunks per superchunk (ACT table-set batch)
NSUPER = NCHUNK // K  # 4

# chunks whose reciprocal runs on DVE (Newton) instead of ACT
DVE_RECIP = frozenset({8, 9, 10, 11, 13})
# msg PSUM->SBUF copy engine per batch: True -> ACT, False -> DVE
MSG_ON_ACT = tuple(False for _ in range(NCHUNK))

SQ2 = 1.4142135623730951
WSCALE = 0.8862269254527580  # sqrt(pi)/2, cancels D_Erf's 2/sqrt(pi)
# fp16 bitwise-NOT reciprocal seed + 1 Newton: y1 = c0*s*(c1 - r*c0*s),
# s = bitcast16(~bits16(r)). Constants minimax-fit over [0.05, 2.05].
NR_C0 = -0.23563272
NR_C1 = 2.00172757

_CACHE = {}


def _install_ntff_hook():
    """Provide antenv.axon_hooks (absent in this image) so trace=True can
    NTFF-profile through libaxon. Only needed for profiling runs."""
    if "antenv.axon_hooks" in sys.modules:
        return
    import antenv

    mod = types.ModuleType("antenv.axon_hooks")
    state = {"hook": None}
    mod.set_axon_ntff_profile_hook = lambda h: state.__setitem__("hook", h)
    mod.get_axon_ntff_profile_hook = lambda: state["hook"]
    sys.modules["antenv.axon_hooks"] = mod
    antenv.axon_hooks = mod
    try:
        from trn_agent_boot.trn_boot import _ntff_profile_via_ctypes

        mod.set_axon_ntff_profile_hook(
            _ntff_profile_via_ctypes("/opt/axon/libaxon_pjrt.so")
        )
    except Exception:
        pass


def _split_excess_waits(nc, max_waits=1):
    """Walrus here encodes at most one sync-wait per TPB instruction.
    Hoist extras onto preceding same-engine wait-only carriers."""
    import bass_rust

    seq = 0
    for fn in nc.m.functions:
        for bb in fn.blocks:
            insts = list(bb.instructions)
            out = []
            dirty = False
            for inst in insts:
                si = inst.sync_info
                if si is None:
                    out.append(inst)
                    continue
                waits = list(si.on_wait)
                if len(waits) > max_waits:
                    for w in waits[:-max_waits]:
                        seq += 1
                        carrier = bass_rust.InstEventSemaphore(
                            name=f"WSPLIT-{seq}", ins=[], outs=[]
                        )
                        carrier.engine = inst.engine
                        carrier.sync_info = bass_rust.SyncInfo(
                            on_wait=[w], on_update=[]
                        )
                        out.append(carrier)
                    inst.sync_info = bass_rust.SyncInfo(
                        on_wait=waits[-max_waits:], on_update=list(si.on_update)
                    )
                    dirty = True
                out.append(inst)
            if dirty:
                bb.instructions = out
    return seq


def _build(has_bias):
    import concourse.bass as bass
    import concourse.tile as tile
    from concourse import mybir

    f32 = mybir.dt.float32
    f16 = mybir.dt.float16
    u16 = mybir.dt.uint16
    AF = mybir.ActivationFunctionType
    OP = mybir.AluOpType

    nc = bass.Bass(
        "TRN2", target_bir_lowering=False, debug=False, num_devices=NCORES
    )
    distT_d = nc.dram_tensor("distT", [N, ROWS], f16, kind="ExternalInput").ap()
    zT_d = nc.dram_tensor("zT", [D, N], f16, kind="ExternalInput").ap()
    w_d = nc.dram_tensor("w", [D, D], f16, kind="ExternalInput").ap()
    b_d = nc.dram_tensor("b", [1, D], f16, kind="ExternalInput").ap()
    outT_d = nc.dram_tensor("outT", [D, ROWS], f32, kind="ExternalOutput").ap()

    def act_raw(out_ap, in_ap, func, bias, scale):
        return nc.scalar.add_instruction(
            mybir.InstActivation(
                name=nc.get_next_instruction_name(),
                func=func,
                ins=[
                    nc.scalar.lower_ap(in_ap),
                    mybir.ImmediateValue(dtype=f32, value=bias),
                    mybir.ImmediateValue(dtype=f32, value=scale),
                    mybir.ImmediateValue(dtype=f32, value=0.0),
                ],
                outs=[nc.scalar.lower_ap(out_ap)],
            )
        )

    with tile.TileContext(nc) as tc:
        with (
            tc.tile_pool(name="wb", bufs=1) as wbpool,
            tc.tile_pool(name="zq", bufs=1) as zqpool,
            tc.tile_pool(name="msgp", bufs=1) as msgpool,
            tc.tile_pool(name="msgps", bufs=2, space="PSUM") as msgpsum,
            tc.tile_pool(name="rch", bufs=3) as rpool,
            tc.tile_pool(name="mch", bufs=3) as mpool,
            tc.tile_pool(name="tch", bufs=5) as tpool,
            tc.tile_pool(name="nsc", bufs=1) as nscpool,
            tc.tile_pool(name="wch", bufs=2) as wpool,
            tc.tile_pool(name="outps", bufs=1, space="PSUM") as outpsum,
            tc.tile_pool(name="outsb", bufs=1) as outpool,
        ):
            # ---- W (fp16, host-scaled), optional bias, zT halves ----
            w_sb = [
                wbpool.tile([JT, D], f16, tag=f"wsb{h}", name=f"wsb{h}")
                for h in (0, 1)
            ]
            zq = [
                zqpool.tile([JT, N], f16, tag=f"zq{h}", name=f"zq{h}")
                for h in (0, 1)
            ]

            def load_z_quarter(q):
                for h in (0, 1):
                    nc.sync.dma_start(
                        zq[h][:, q * 2048 : (q + 1) * 2048],
                        zT_d[h * JT : (h + 1) * JT, q * 2048 : (q + 1) * 2048],
                    )

            def load_wb():
                for h in (0, 1):
                    nc.sync.dma_start(w_sb[h][:], w_d[h * JT : (h + 1) * JT, :])
                if has_bias:
                    nc.sync.dma_start(b_sb[:], b_d[:])
                    nc.gpsimd.memset(ones[:], 1.0)

            if has_bias:
                b_sb = wbpool.tile([1, D], f16)
                ones = wbpool.tile([1, JT], f16)

            msg = [
                msgpool.tile([JT, CHUNK_JT * D], f16, name=f"msg{b}", tag=f"msg{b}")
                for b in range(NCHUNK)
            ]
            acc = [
                outpsum.tile([JT, ROWS], f32, tag=f"acc{h}", name=f"acc{h}")
                for h in (0, 1)
            ]

            def emit_msg_matmuls(B):
                ps = msgpsum.tile([JT, CHUNK_JT * D], f32, name=f"mps{B}", tag="mps")
                for jj in range(CHUNK_JT):
                    jg = B * CHUNK_JT + jj
                    pslice = ps[:, jj * D : (jj + 1) * D]
                    for h in (0, 1):
                        nc.tensor.matmul(
                            pslice,
                            zq[h][:, jg * JT : (jg + 1) * JT],
                            w_sb[h][:],
                            start=(h == 0),
                            stop=(h == 1 and not has_bias),
                        )
                    if has_bias:
                        nc.tensor.matmul(
                            pslice, ones[:], b_sb[:], start=False, stop=True
                        )
                return ps

            def emit_msg_cast(B, ps):
                if MSG_ON_ACT[B]:
                    nc.scalar.copy(msg[B][:], ps[:])
                else:
                    nc.vector.tensor_copy(msg[B][:], ps[:])

            def emit_r_dma(c):
                r = rpool.tile([JT, CHUNK_F], f16, name=f"r{c}", tag="r")
                for k in range(CHUNK_JT):
                    jt = c * CHUNK_JT + k
                    nc.sync.dma_start(
                        r[:, k * ROWS : (k + 1) * ROWS],
                        distT_d[jt * JT : (jt + 1) * JT, :],
                    )
                return r

            def emit_p1_dve(c, r):
                """DVE-side phase 1: mask make, plus the Newton reciprocal
                for DVE_RECIP chunks."""
                m = mpool.tile([JT, CHUNK_F], f16, name=f"m{c}", tag="m")
                nc.vector.tensor_scalar(
                    m[:], r[:], 1.0, 60000.0, op0=OP.is_ge, op1=OP.mult
                )
                if c in DVE_RECIP:
                    # s = bitcast(~r); p = r*s (in place over the dead r);
                    # q = (p - c1/c0)*(-c0^2); t = s*q == c0*s*(c1 - r*c0*s)
                    t = tpool.tile([JT, CHUNK_F], f16, name=f"t{c}", tag="t")
                    s_t = nscpool.tile([JT, CHUNK_F], u16, name=f"ns{c}", tag="ns")
                    nc.vector.tensor_scalar(
                        s_t[:], r[:].bitcast(u16), 0, None, op0=OP.bitwise_not
                    )
                    nc.vector.tensor_tensor(
                        r[:], r[:], s_t[:].bitcast(f16), op=OP.mult
                    )
                    nc.vector.tensor_scalar(
                        r[:], r[:], NR_C1 / NR_C0, -NR_C0 * NR_C0,
                        op0=OP.subtract, op1=OP.mult,
                    )
                    nc.vector.tensor_tensor(
                        t[:], s_t[:].bitcast(f16), r[:], op=OP.mult
                    )
                    return c, None, t, m
                return c, r, None, m

            def emit_p1_act(entry):
                """ACT-side phase 1: the raw Reciprocal for ACT chunks."""
                c, r, t, m = entry
                if t is None:
                    t = tpool.tile([JT, CHUNK_F], f16, name=f"t{c}", tag="t")
                    act_raw(t[:], r[:], AF.Reciprocal, 0.0, 1.0)
                return c, t, m

            def emit_apply(c, t, m):
                # masked elements -> 60000; D_Erf saturates them to exact 0
                nc.vector.tensor_tensor(t[:], t[:], m[:], op=OP.max)

            def emit_derf_pe(c, t):
                w = wpool.tile([JT, CHUNK_F], f16, name=f"w{c}", tag="w")
                act_raw(w[:], t[:], AF.Derivative_Erf, -SQ2, SQ2)
                for k in range(CHUNK_JT):
                    jt = c * CHUNK_JT + k
                    mtile = msg[c]
                    for h in (0, 1):
                        lhsT = mtile[:, k * D + h * JT : k * D + (h + 1) * JT]
                        for nh in (0, 1):
                            nc.tensor.matmul(
                                acc[h][:, nh * 512 : (nh + 1) * 512],
                                lhsT,
                                w[:, k * ROWS + nh * 512 : k * ROWS + (nh + 1) * 512],
                                start=(jt == 0),
                                stop=(jt == NJT - 1),
                            )

            # ---- interleaved emission ----
            # Per-engine program orders (in-order engines!):
            #   DVE: apply(c), cast(c'), make(c'), newton(c') interleaved
            #        per chunk, so applies never queue behind a whole
            #        super's phase-1 burst.
            #   ACT: [derf run (D table)] then [recip run (R table)] per
            #        super: 2 table loads per super, and derfs aren't
            #        blocked behind recips that wait on fresh DMA.
            def emit_phase1(c):
                r = emit_r_dma(c)
                return emit_p1_act(emit_p1_dve(c, r))

            def emit_phase2(c, t, m):
                emit_apply(c, t, m)
                emit_derf_pe(c, t)

            pending = [emit_phase1(0)]
            load_wb()
            load_z_quarter(0)
            pending += [emit_phase1(c) for c in range(1, K)]
            for s in range(NSUPER):
                for B in range(K * s, K * s + K):
                    emit_msg_cast(B, emit_msg_matmuls(B))
                if s + 1 < NSUPER:
                    load_z_quarter(s + 1)
                    nxt = [emit_phase1((s + 1) * K + k) for k in range(K)]
                else:
                    nxt = []
                for k in range(K):
                    emit_phase2(*pending[k])
                pending = nxt

            # ---- tail: PSUM -> SBUF fp32 -> HBM ----
            for h in (0, 1):
                o = outpool.tile([JT, ROWS], f32, tag=f"o{h}", name=f"o{h}")
                nc.vector.tensor_copy(o[:], acc[h][:])
                nc.sync.dma_start(outT_d[h * JT : (h + 1) * JT, :], o[:])

    _split_excess_waits(nc)
    return nc


def kernel(z, dist_matrix, W, B, _trace=False):
    from concourse.bass_utils import run_bass_kernel_spmd

    if _trace:
        _install_ntff_hook()

    dist = np.asarray(dist_matrix, np.float32)
    z = np.asarray(z, np.float32)
    W_np = np.asarray(W, np.float32)
    B_np = np.asarray(B, np.float32).reshape(1, D)
    has_bias = bool(np.any(B_np))

    key = ("nc", has_bias)
    if key not in _CACHE:
        _CACHE[key] = _build(has_bias)
    nc = _CACHE[key]

    # fp16 dist with an exact cutoff: values < 1 that round UP to 1.0
    # would flip the mask; pin them to the largest fp16 below 1.
    r16 = dist.astype(np.float16)
    bad = (dist < 1.0) & (r16 >= 1.0)
    if bad.any():
        r16[bad] = np.float16(0.99951171875)

    zT16 = np.ascontiguousarray(z.T.astype(np.float16))
    W16 = (W_np * WSCALE).astype(np.float16)
    B16 = (B_np * WSCALE).astype(np.float16)

    in_maps = []
    for c in range(NCORES):
        blk = np.ascontiguousarray(r16[c * ROWS : (c + 1) * ROWS, :].T)
        in_maps.append({"distT": blk, "zT": zT16, "w": W16, "b": B16})

    res = run_bass_kernel_spmd(
        nc, in_maps, core_ids=list(range(NCORES)), trace=_trace
    )
    _CACHE["last"] = res

    out = np.empty((N, D), np.float32)
    for c in range(NCORES):
        out[c * ROWS : (c + 1) * ROWS, :] = res.results[c]["outT"].T
    return out


# revision 27
# speedup vs baseline: 1.2857x; 1.0272x over previous
"""nn_InteractionLayer Bass/Tile kernel for 8 Trainium2 NeuronCores.

out = where(dist < 1, exp(-2*(1/dist - 1)^2), 0) @ (z @ W + B)
N = 8192, D = 256.

Row-parallel: core c owns rows [c*1024, (c+1)*1024) of dist_matrix.
dist is shipped host-side as fp16 (halves the dominant HBM read) in
transposed [j, i] layout; a host boundary fix keeps the r<1 cutoff
bit-exact across the fp16 rounding. z is shipped transposed+fp16.

Per-core dataflow, 16 chunks of [128, 4096] (4 j-tiles):
  msg   = z @ (W*sqrt(pi)/2)  [N, D] fp16 via 16 PSUM batches (bias
          matmuls only if B is nonzero; B is zero in this problem).
          The sqrt(pi)/2 pre-scale host-side cancels Derivative_Erf's
          2/sqrt(pi) factor.
  m     = (r >= 1) * 60000          DVE ts chain (4x mode)
  t     = 1/r:  ACT raw Reciprocal for chunks in ACT_RECIP, else a
          DVE stock-op Newton chain (bitwise-NOT seed on the fp16 bit
          pattern + 1 Newton step; ~2.6e-3 rel err, plenty for the
          2e-2 gate) to offload the saturated ACT engine.
  t'    = max(t, m)                 DVE tt (masked elems -> 60000)
  w     = Derivative_Erf(sqrt2*t' - sqrt2) = (2/sqrt(pi))*exp(-2(t'-1)^2)
          ACT, immediate scale/bias; masked input saturates to exact 0.
  outT[d, i] += msg_chunk^T @ w     PE, PSUM fp32, 4 banks

ACT table sets are batched per 4-chunk super (recips then previous
super's D_Erfs) to bound table reloads. Reciprocal/Derivative_Erf are
emitted as raw InstActivation (wrapper vetoes Reciprocal on accuracy
grounds; measured ~1e-5 rel here, tolerance 2e-2).

This container's walrus encodes at most ONE semaphore wait per TPB
instruction; a post-Tile pass splits extra waits onto same-engine
EventSemaphore carriers (semantically identical, program order).
"""

import sys
import types

if "/opt/trn_rl_repo" not in sys.path:
    sys.path.insert(0, "/opt/trn_rl_repo")

import numpy as np

N = 8192
D = 256
NCORES = 8
ROWS = N // NCORES  # 1024 rows of dist per core
JT = 128  # j-tile (partition dim)
NJT = N // JT  # 64 j-tiles
CHUNK_JT = 4  # j-tiles per elementwise chunk
CHUNK_F = CHUNK_JT * ROWS  # free-dim elements per chunk tile (4096)
NCHUNK = NJT // CHUNK_JT  # 16
K = 4  # chunks per superchunk (ACT table-set batch)
NSUPER = NCHUNK // K  # 4

# chunks whose reciprocal runs on DVE (Newton) instead of ACT
DVE_RECIP = frozenset({8, 9, 10, 11, 13})
# msg PSUM->SBUF copy engine per batch: True -> ACT, False -> DVE.
# Batches 8-11 ride ACT: their super has no ACT recips (all-DVE Newton),
# so ACT has idle capacity there while DVE is newton-heavy.
MSG_ON_ACT = tuple(c in (8, 9, 10, 11) for c in range(NCHUNK))

SQ2 = 1.4142135623730951
WSCALE = 0.8862269254527580  # sqrt(pi)/2, cancels D_Erf's 2/sqrt(pi)
# fp16 bitwise-NOT reciprocal seed + 1 Newton: y1 = c0*s*(c1 - r*c0*s),
# s = bitcast16(~bits16(r)). Constants minimax-fit over [0.05, 2.05].
NR_C0 = -0.23563272
NR_C1 = 2.00172757

_CACHE = {}


def _install_ntff_hook():
    """Provide antenv.axon_hooks (absent in this image) so trace=True can
    NTFF-profile through libaxon. Only needed for profiling runs."""
    if "antenv.axon_hooks" in sys.modules:
        return
    import antenv

    mod = types.ModuleType("antenv.axon_hooks")
    state = {"hook": None}
    mod.set_axon_ntff_profile_hook = lambda h: state.__setitem__("hook", h)
    mod.get_axon_ntff_profile_hook = lambda: state["hook"]
    sys.modules["antenv.axon_hooks"] = mod
    antenv.axon_hooks = mod
    try:
        from trn_agent_boot.trn_boot import _ntff_profile_via_ctypes

        mod.set_axon_ntff_profile_hook(
            _ntff_profile_via_ctypes("/opt/axon/libaxon_pjrt.so")
        )
    except Exception:
        pass


def _split_excess_waits(nc, max_waits=1):
    """Walrus here encodes at most one sync-wait per TPB instruction.
    Hoist extras onto preceding same-engine wait-only carriers."""
    import bass_rust

    seq = 0
    for fn in nc.m.functions:
        for bb in fn.blocks:
            insts = list(bb.instructions)
            out = []
            dirty = False
            for inst in insts:
                si = inst.sync_info
                if si is None:
                    out.append(inst)
                    continue
                waits = list(si.on_wait)
                if len(waits) > max_waits:
                    for w in waits[:-max_waits]:
                        seq += 1
                        carrier = bass_rust.InstEventSemaphore(
                            name=f"WSPLIT-{seq}", ins=[], outs=[]
                        )
                        carrier.engine = inst.engine
                        carrier.sync_info = bass_rust.SyncInfo(
                            on_wait=[w], on_update=[]
                        )
                        out.append(carrier)
                    inst.sync_info = bass_rust.SyncInfo(
                        on_wait=waits[-max_waits:], on_update=list(si.on_update)
                    )
                    dirty = True
                out.append(inst)
            if dirty:
                bb.instructions = out
    return seq


def _build(has_bias):
    import concourse.bass as bass
    import concourse.tile as tile
    from concourse import mybir

    f32 = mybir.dt.float32
    f16 = mybir.dt.float16
    u16 = mybir.dt.uint16
    AF = mybir.ActivationFunctionType
    OP = mybir.AluOpType

    nc = bass.Bass(
        "TRN2", target_bir_lowering=False, debug=False, num_devices=NCORES
    )
    distT_d = nc.dram_tensor("distT", [N, ROWS], f16, kind="ExternalInput").ap()
    zT_d = nc.dram_tensor("zT", [D, N], f16, kind="ExternalInput").ap()
    w_d = nc.dram_tensor("w", [D, D], f16, kind="ExternalInput").ap()
    b_d = nc.dram_tensor("b", [1, D], f16, kind="ExternalInput").ap()
    outT_d = nc.dram_tensor("outT", [D, ROWS], f32, kind="ExternalOutput").ap()

    def act_raw(out_ap, in_ap, func, bias, scale):
        return nc.scalar.add_instruction(
            mybir.InstActivation(
                name=nc.get_next_instruction_name(),
                func=func,
                ins=[
                    nc.scalar.lower_ap(in_ap),
                    mybir.ImmediateValue(dtype=f32, value=bias),
                    mybir.ImmediateValue(dtype=f32, value=scale),
                    mybir.ImmediateValue(dtype=f32, value=0.0),
                ],
                outs=[nc.scalar.lower_ap(out_ap)],
            )
        )

    with tile.TileContext(nc) as tc:
        with (
            tc.tile_pool(name="wb", bufs=1) as wbpool,
            tc.tile_pool(name="zq", bufs=1) as zqpool,
            tc.tile_pool(name="msgp", bufs=1) as msgpool,
            tc.tile_pool(name="msgps", bufs=2, space="PSUM") as msgpsum,
            tc.tile_pool(name="rch", bufs=3) as rpool,
            tc.tile_pool(name="mch", bufs=3) as mpool,
            tc.tile_pool(name="tch", bufs=5) as tpool,
            tc.tile_pool(name="nsc", bufs=1) as nscpool,
            tc.tile_pool(name="wch", bufs=2) as wpool,
            tc.tile_pool(name="outps", bufs=1, space="PSUM") as outpsum,
            tc.tile_pool(name="outsb", bufs=1) as outpool,
        ):
            # ---- W (fp16, host-scaled), optional bias, zT halves ----
            w_sb = [
                wbpool.tile([JT, D], f16, tag=f"wsb{h}", name=f"wsb{h}")
                for h in (0, 1)
            ]
            zq = [
                zqpool.tile([JT, N], f16, tag=f"zq{h}", name=f"zq{h}")
                for h in (0, 1)
            ]

            def load_z_quarter(q):
                for h in (0, 1):
                    nc.sync.dma_start(
                        zq[h][:, q * 2048 : (q + 1) * 2048],
                        zT_d[h * JT : (h + 1) * JT, q * 2048 : (q + 1) * 2048],
                    )

            def load_wb():
                for h in (0, 1):
                    nc.sync.dma_start(w_sb[h][:], w_d[h * JT : (h + 1) * JT, :])
                if has_bias:
                    nc.sync.dma_start(b_sb[:], b_d[:])
                    nc.gpsimd.memset(ones[:], 1.0)

            if has_bias:
                b_sb = wbpool.tile([1, D], f16)
                ones = wbpool.tile([1, JT], f16)

            msg = [
                msgpool.tile([JT, CHUNK_JT * D], f16, name=f"msg{b}", tag=f"msg{b}")
                for b in range(NCHUNK)
            ]
            acc = [
                outpsum.tile([JT, ROWS], f32, tag=f"acc{h}", name=f"acc{h}")
                for h in (0, 1)
            ]

            def emit_msg_matmuls(B):
                ps = msgpsum.tile([JT, CHUNK_JT * D], f32, name=f"mps{B}", tag="mps")
                for jj in range(CHUNK_JT):
                    jg = B * CHUNK_JT + jj
                    pslice = ps[:, jj * D : (jj + 1) * D]
                    for h in (0, 1):
                        nc.tensor.matmul(
                            pslice,
                            zq[h][:, jg * JT : (jg + 1) * JT],
                            w_sb[h][:],
                            start=(h == 0),
                            stop=(h == 1 and not has_bias),
                        )
                    if has_bias:
                        nc.tensor.matmul(
                            pslice, ones[:], b_sb[:], start=False, stop=True
                        )
                return ps

            def emit_msg_cast(B, ps):
                if MSG_ON_ACT[B]:
                    nc.scalar.copy(msg[B][:], ps[:])
                else:
                    nc.vector.tensor_copy(msg[B][:], ps[:])

            def emit_r_dma(c):
                r = rpool.tile([JT, CHUNK_F], f16, name=f"r{c}", tag="r")
                for k in range(CHUNK_JT):
                    jt = c * CHUNK_JT + k
                    nc.sync.dma_start(
                        r[:, k * ROWS : (k + 1) * ROWS],
                        distT_d[jt * JT : (jt + 1) * JT, :],
                    )
                return r

            def emit_p1_dve(c, r):
                """DVE-side phase 1: mask make, plus the Newton reciprocal
                for DVE_RECIP chunks."""
                m = mpool.tile([JT, CHUNK_F], f16, name=f"m{c}", tag="m")
                nc.vector.tensor_scalar(
                    m[:], r[:], 1.0, 60000.0, op0=OP.is_ge, op1=OP.mult
                )
                if c in DVE_RECIP:
                    # s = bitcast(~r); p = r*s (in place over the dead r);
                    # q = (p - c1/c0)*(-c0^2); t = s*q == c0*s*(c1 - r*c0*s)
                    t = tpool.tile([JT, CHUNK_F], f16, name=f"t{c}", tag="t")
                    s_t = nscpool.tile([JT, CHUNK_F], u16, name=f"ns{c}", tag="ns")
                    nc.vector.tensor_scalar(
                        s_t[:], r[:].bitcast(u16), 0, None, op0=OP.bitwise_not
                    )
                    nc.vector.tensor_tensor(
                        r[:], r[:], s_t[:].bitcast(f16), op=OP.mult
                    )
                    nc.vector.tensor_scalar(
                        r[:], r[:], NR_C1 / NR_C0, -NR_C0 * NR_C0,
                        op0=OP.subtract, op1=OP.mult,
                    )
                    nc.vector.tensor_tensor(
                        t[:], s_t[:].bitcast(f16), r[:], op=OP.mult
                    )
                    return c, None, t, m
                return c, r, None, m

            def emit_p1_act(entry):
                """ACT-side phase 1: the raw Reciprocal for ACT chunks."""
                c, r, t, m = entry
                if t is None:
                    t = tpool.tile([JT, CHUNK_F], f16, name=f"t{c}", tag="t")
                    act_raw(t[:], r[:], AF.Reciprocal, 0.0, 1.0)
                return c, t, m

            def emit_apply(c, t, m):
                # masked elements -> 60000; D_Erf saturates them to exact 0
                nc.vector.tensor_tensor(t[:], t[:], m[:], op=OP.max)

            def emit_derf_pe(c, t):
                w = wpool.tile([JT, CHUNK_F], f16, name=f"w{c}", tag="w")
                act_raw(w[:], t[:], AF.Derivative_Erf, -SQ2, SQ2)
                for k in range(CHUNK_JT):
                    jt = c * CHUNK_JT + k
                    mtile = msg[c]
                    for h in (0, 1):
                        lhsT = mtile[:, k * D + h * JT : k * D + (h + 1) * JT]
                        for nh in (0, 1):
                            nc.tensor.matmul(
                                acc[h][:, nh * 512 : (nh + 1) * 512],
                                lhsT,
                                w[:, k * ROWS + nh * 512 : k * ROWS + (nh + 1) * 512],
                                start=(jt == 0),
                                stop=(jt == NJT - 1),
                            )

            # ---- interleaved emission ----
            # Per-engine program orders (in-order engines!):
            #   DVE: apply(c), cast(c'), make(c'), newton(c') interleaved
            #        per chunk, so applies never queue behind a whole
            #        super's phase-1 burst.
            #   ACT: [derf run (D table)] then [recip run (R table)] per
            #        super: 2 table loads per super, and derfs aren't
            #        blocked behind recips that wait on fresh DMA.
            def emit_phase1(c):
                r = emit_r_dma(c)
                return emit_p1_act(emit_p1_dve(c, r))

            def emit_phase2(c, t, m):
                emit_apply(c, t, m)
                emit_derf_pe(c, t)

            pending = [emit_phase1(0)]
            load_wb()
            load_z_quarter(0)
            pending += [emit_phase1(c) for c in range(1, K)]
            for s in range(NSUPER):
                for B in range(K * s, K * s + K):
                    emit_msg_cast(B, emit_msg_matmuls(B))
                if s + 1 < NSUPER:
                    load_z_quarter(s + 1)
                    nxt = [emit_phase1((s + 1) * K + k) for k in range(K)]
                else:
                    nxt = []
                for k in range(K):
                    emit_phase2(*pending[k])
                pending = nxt

            # ---- tail: PSUM -> SBUF fp32 -> HBM ----
            for h in (0, 1):
                o = outpool.tile([JT, ROWS], f32, tag=f"o{h}", name=f"o{h}")
                nc.vector.tensor_copy(o[:], acc[h][:])
                nc.sync.dma_start(outT_d[h * JT : (h + 1) * JT, :], o[:])

    _split_excess_waits(nc)
    return nc


def kernel(z, dist_matrix, W, B, _trace=False):
    from concourse.bass_utils import run_bass_kernel_spmd

    if _trace:
        _install_ntff_hook()

    dist = np.asarray(dist_matrix, np.float32)
    z = np.asarray(z, np.float32)
    W_np = np.asarray(W, np.float32)
    B_np = np.asarray(B, np.float32).reshape(1, D)
    has_bias = bool(np.any(B_np))

    key = ("nc", has_bias)
    if key not in _CACHE:
        _CACHE[key] = _build(has_bias)
    nc = _CACHE[key]

    # fp16 dist with an exact cutoff: values < 1 that round UP to 1.0
    # would flip the mask; pin them to the largest fp16 below 1.
    r16 = dist.astype(np.float16)
    bad = (dist < 1.0) & (r16 >= 1.0)
    if bad.any():
        r16[bad] = np.float16(0.99951171875)

    zT16 = np.ascontiguousarray(z.T.astype(np.float16))
    W16 = (W_np * WSCALE).astype(np.float16)
    B16 = (B_np * WSCALE).astype(np.float16)

    in_maps = []
    for c in range(NCORES):
        blk = np.ascontiguousarray(r16[c * ROWS : (c + 1) * ROWS, :].T)
        in_maps.append({"distT": blk, "zT": zT16, "w": W16, "b": B16})

    res = run_bass_kernel_spmd(
        nc, in_maps, core_ids=list(range(NCORES)), trace=_trace
    )
    _CACHE["last"] = res

    out = np.empty((N, D), np.float32)
    for c in range(NCORES):
        out[c * ROWS : (c + 1) * ROWS, :] = res.results[c]["outT"].T
    return out


# revision 28
# speedup vs baseline: 1.2891x; 1.0027x over previous
"""nn_InteractionLayer Bass/Tile kernel for 8 Trainium2 NeuronCores.

out = where(dist < 1, exp(-2*(1/dist - 1)^2), 0) @ (z @ W + B)
N = 8192, D = 256.

Row-parallel: core c owns rows [c*1024, (c+1)*1024) of dist_matrix.
dist is shipped host-side as fp16 (halves the dominant HBM read) in
transposed [j, i] layout; a host boundary fix keeps the r<1 cutoff
bit-exact across the fp16 rounding. z is shipped transposed+fp16.

Per-core dataflow, 16 chunks of [128, 4096] (4 j-tiles):
  msg   = z @ (W*sqrt(pi)/2)  [N, D] fp16 via 16 PSUM batches (bias
          matmuls only if B is nonzero; B is zero in this problem).
          The sqrt(pi)/2 pre-scale host-side cancels Derivative_Erf's
          2/sqrt(pi) factor.
  m     = (r >= 1) * 60000          DVE ts chain (4x mode)
  t     = 1/r:  ACT raw Reciprocal for chunks in ACT_RECIP, else a
          DVE stock-op Newton chain (bitwise-NOT seed on the fp16 bit
          pattern + 1 Newton step; ~2.6e-3 rel err, plenty for the
          2e-2 gate) to offload the saturated ACT engine.
  t'    = max(t, m)                 DVE tt (masked elems -> 60000)
  w     = Derivative_Erf(sqrt2*t' - sqrt2) = (2/sqrt(pi))*exp(-2(t'-1)^2)
          ACT, immediate scale/bias; masked input saturates to exact 0.
  outT[d, i] += msg_chunk^T @ w     PE, PSUM fp32, 4 banks

ACT table sets are batched per 4-chunk super (recips then previous
super's D_Erfs) to bound table reloads. Reciprocal/Derivative_Erf are
emitted as raw InstActivation (wrapper vetoes Reciprocal on accuracy
grounds; measured ~1e-5 rel here, tolerance 2e-2).

This container's walrus encodes at most ONE semaphore wait per TPB
instruction; a post-Tile pass splits extra waits onto same-engine
EventSemaphore carriers (semantically identical, program order).
"""

import sys
import types

if "/opt/trn_rl_repo" not in sys.path:
    sys.path.insert(0, "/opt/trn_rl_repo")

import numpy as np

N = 8192
D = 256
NCORES = 8
ROWS = N // NCORES  # 1024 rows of dist per core
JT = 128  # j-tile (partition dim)
NJT = N // JT  # 64 j-tiles
CHUNK_JT = 4  # j-tiles per elementwise chunk
CHUNK_F = CHUNK_JT * ROWS  # free-dim elements per chunk tile (4096)
NCHUNK = NJT // CHUNK_JT  # 16
K = 4  # chunks per superchunk (ACT table-set batch)
NSUPER = NCHUNK // K  # 4

# chunks whose reciprocal runs on DVE (Newton) instead of ACT
DVE_RECIP = frozenset({8, 9, 10, 11, 13})
# msg PSUM->SBUF copy engine per batch: True -> ACT, False -> DVE.
# Batches 8-11 ride ACT: their super has no ACT recips (all-DVE Newton),
# so ACT has idle capacity there while DVE is newton-heavy.
MSG_ON_ACT = tuple(c in (8, 9, 10, 11, 12, 13, 14, 15) for c in range(NCHUNK))

SQ2 = 1.4142135623730951
WSCALE = 0.8862269254527580  # sqrt(pi)/2, cancels D_Erf's 2/sqrt(pi)
# fp16 bitwise-NOT reciprocal seed + 1 Newton: y1 = c0*s*(c1 - r*c0*s),
# s = bitcast16(~bits16(r)). Constants minimax-fit over [0.05, 2.05].
NR_C0 = -0.23563272
NR_C1 = 2.00172757

_CACHE = {}


def _install_ntff_hook():
    """Provide antenv.axon_hooks (absent in this image) so trace=True can
    NTFF-profile through libaxon. Only needed for profiling runs."""
    if "antenv.axon_hooks" in sys.modules:
        return
    import antenv

    mod = types.ModuleType("antenv.axon_hooks")
    state = {"hook": None}
    mod.set_axon_ntff_profile_hook = lambda h: state.__setitem__("hook", h)
    mod.get_axon_ntff_profile_hook = lambda: state["hook"]
    sys.modules["antenv.axon_hooks"] = mod
    antenv.axon_hooks = mod
    try:
        from trn_agent_boot.trn_boot import _ntff_profile_via_ctypes

        mod.set_axon_ntff_profile_hook(
            _ntff_profile_via_ctypes("/opt/axon/libaxon_pjrt.so")
        )
    except Exception:
        pass


def _split_excess_waits(nc, max_waits=1):
    """Walrus here encodes at most one sync-wait per TPB instruction.
    Hoist extras onto preceding same-engine wait-only carriers."""
    import bass_rust

    seq = 0
    for fn in nc.m.functions:
        for bb in fn.blocks:
            insts = list(bb.instructions)
            out = []
            dirty = False
            for inst in insts:
                si = inst.sync_info
                if si is None:
                    out.append(inst)
                    continue
                waits = list(si.on_wait)
                if len(waits) > max_waits:
                    for w in waits[:-max_waits]:
                        seq += 1
                        carrier = bass_rust.InstEventSemaphore(
                            name=f"WSPLIT-{seq}", ins=[], outs=[]
                        )
                        carrier.engine = inst.engine
                        carrier.sync_info = bass_rust.SyncInfo(
                            on_wait=[w], on_update=[]
                        )
                        out.append(carrier)
                    inst.sync_info = bass_rust.SyncInfo(
                        on_wait=waits[-max_waits:], on_update=list(si.on_update)
                    )
                    dirty = True
                out.append(inst)
            if dirty:
                bb.instructions = out
    return seq


def _build(has_bias):
    import concourse.bass as bass
    import concourse.tile as tile
    from concourse import mybir

    f32 = mybir.dt.float32
    f16 = mybir.dt.float16
    u16 = mybir.dt.uint16
    AF = mybir.ActivationFunctionType
    OP = mybir.AluOpType

    nc = bass.Bass(
        "TRN2", target_bir_lowering=False, debug=False, num_devices=NCORES
    )
    distT_d = nc.dram_tensor("distT", [N, ROWS], f16, kind="ExternalInput").ap()
    zT_d = nc.dram_tensor("zT", [D, N], f16, kind="ExternalInput").ap()
    w_d = nc.dram_tensor("w", [D, D], f16, kind="ExternalInput").ap()
    b_d = nc.dram_tensor("b", [1, D], f16, kind="ExternalInput").ap()
    outT_d = nc.dram_tensor("outT", [D, ROWS], f32, kind="ExternalOutput").ap()

    def act_raw(out_ap, in_ap, func, bias, scale):
        return nc.scalar.add_instruction(
            mybir.InstActivation(
                name=nc.get_next_instruction_name(),
                func=func,
                ins=[
                    nc.scalar.lower_ap(in_ap),
                    mybir.ImmediateValue(dtype=f32, value=bias),
                    mybir.ImmediateValue(dtype=f32, value=scale),
                    mybir.ImmediateValue(dtype=f32, value=0.0),
                ],
                outs=[nc.scalar.lower_ap(out_ap)],
            )
        )

    with tile.TileContext(nc) as tc:
        with (
            tc.tile_pool(name="wb", bufs=1) as wbpool,
            tc.tile_pool(name="zq", bufs=1) as zqpool,
            tc.tile_pool(name="msgp", bufs=1) as msgpool,
            tc.tile_pool(name="msgps", bufs=2, space="PSUM") as msgpsum,
            tc.tile_pool(name="rch", bufs=3) as rpool,
            tc.tile_pool(name="mch", bufs=3) as mpool,
            tc.tile_pool(name="tch", bufs=5) as tpool,
            tc.tile_pool(name="nsc", bufs=1) as nscpool,
            tc.tile_pool(name="wch", bufs=2) as wpool,
            tc.tile_pool(name="outps", bufs=1, space="PSUM") as outpsum,
            tc.tile_pool(name="outsb", bufs=1) as outpool,
        ):
            # ---- W (fp16, host-scaled), optional bias, zT halves ----
            w_sb = [
                wbpool.tile([JT, D], f16, tag=f"wsb{h}", name=f"wsb{h}")
                for h in (0, 1)
            ]
            zq = [
                zqpool.tile([JT, N], f16, tag=f"zq{h}", name=f"zq{h}")
                for h in (0, 1)
            ]

            def load_z_quarter(q):
                for h in (0, 1):
                    nc.sync.dma_start(
                        zq[h][:, q * 2048 : (q + 1) * 2048],
                        zT_d[h * JT : (h + 1) * JT, q * 2048 : (q + 1) * 2048],
                    )

            def load_wb():
                for h in (0, 1):
                    nc.sync.dma_start(w_sb[h][:], w_d[h * JT : (h + 1) * JT, :])
                if has_bias:
                    nc.sync.dma_start(b_sb[:], b_d[:])
                    nc.gpsimd.memset(ones[:], 1.0)

            if has_bias:
                b_sb = wbpool.tile([1, D], f16)
                ones = wbpool.tile([1, JT], f16)

            msg = [
                msgpool.tile([JT, CHUNK_JT * D], f16, name=f"msg{b}", tag=f"msg{b}")
                for b in range(NCHUNK)
            ]
            acc = [
                outpsum.tile([JT, ROWS], f32, tag=f"acc{h}", name=f"acc{h}")
                for h in (0, 1)
            ]

            def emit_msg_matmuls(B):
                ps = msgpsum.tile([JT, CHUNK_JT * D], f32, name=f"mps{B}", tag="mps")
                for jj in range(CHUNK_JT):
                    jg = B * CHUNK_JT + jj
                    pslice = ps[:, jj * D : (jj + 1) * D]
                    for h in (0, 1):
                        nc.tensor.matmul(
                            pslice,
                            zq[h][:, jg * JT : (jg + 1) * JT],
                            w_sb[h][:],
                            start=(h == 0),
                            stop=(h == 1 and not has_bias),
                        )
                    if has_bias:
                        nc.tensor.matmul(
                            pslice, ones[:], b_sb[:], start=False, stop=True
                        )
                return ps

            def emit_msg_cast(B, ps):
                if MSG_ON_ACT[B]:
                    nc.scalar.copy(msg[B][:], ps[:])
                else:
                    nc.vector.tensor_copy(msg[B][:], ps[:])

            def emit_r_dma(c):
                r = rpool.tile([JT, CHUNK_F], f16, name=f"r{c}", tag="r")
                for k in range(CHUNK_JT):
                    jt = c * CHUNK_JT + k
                    nc.sync.dma_start(
                        r[:, k * ROWS : (k + 1) * ROWS],
                        distT_d[jt * JT : (jt + 1) * JT, :],
                    )
                return r

            def emit_p1_dve(c, r):
                """DVE-side phase 1: mask make, plus the Newton reciprocal
                for DVE_RECIP chunks."""
                m = mpool.tile([JT, CHUNK_F], f16, name=f"m{c}", tag="m")
                nc.vector.tensor_scalar(
                    m[:], r[:], 1.0, 60000.0, op0=OP.is_ge, op1=OP.mult
                )
                if c in DVE_RECIP:
                    # s = bitcast(~r); p = r*s (in place over the dead r);
                    # q = (p - c1/c0)*(-c0^2); t = s*q == c0*s*(c1 - r*c0*s)
                    t = tpool.tile([JT, CHUNK_F], f16, name=f"t{c}", tag="t")
                    s_t = nscpool.tile([JT, CHUNK_F], u16, name=f"ns{c}", tag="ns")
                    nc.vector.tensor_scalar(
                        s_t[:], r[:].bitcast(u16), 0, None, op0=OP.bitwise_not
                    )
                    nc.vector.tensor_tensor(
                        r[:], r[:], s_t[:].bitcast(f16), op=OP.mult
                    )
                    nc.vector.tensor_scalar(
                        r[:], r[:], NR_C1 / NR_C0, -NR_C0 * NR_C0,
                        op0=OP.subtract, op1=OP.mult,
                    )
                    nc.vector.tensor_tensor(
                        t[:], s_t[:].bitcast(f16), r[:], op=OP.mult
                    )
                    return c, None, t, m
                return c, r, None, m

            def emit_p1_act(entry):
                """ACT-side phase 1: the raw Reciprocal for ACT chunks."""
                c, r, t, m = entry
                if t is None:
                    t = tpool.tile([JT, CHUNK_F], f16, name=f"t{c}", tag="t")
                    act_raw(t[:], r[:], AF.Reciprocal, 0.0, 1.0)
                return c, t, m

            def emit_apply(c, t, m):
                # masked elements -> 60000; D_Erf saturates them to exact 0
                nc.vector.tensor_tensor(t[:], t[:], m[:], op=OP.max)

            def emit_derf_pe(c, t):
                w = wpool.tile([JT, CHUNK_F], f16, name=f"w{c}", tag="w")
                act_raw(w[:], t[:], AF.Derivative_Erf, -SQ2, SQ2)
                for k in range(CHUNK_JT):
                    jt = c * CHUNK_JT + k
                    mtile = msg[c]
                    for h in (0, 1):
                        lhsT = mtile[:, k * D + h * JT : k * D + (h + 1) * JT]
                        for nh in (0, 1):
                            nc.tensor.matmul(
                                acc[h][:, nh * 512 : (nh + 1) * 512],
                                lhsT,
                                w[:, k * ROWS + nh * 512 : k * ROWS + (nh + 1) * 512],
                                start=(jt == 0),
                                stop=(jt == NJT - 1),
                            )

            # ---- interleaved emission ----
            # Per-engine program orders (in-order engines!):
            #   DVE: apply(c), cast(c'), make(c'), newton(c') interleaved
            #        per chunk, so applies never queue behind a whole
            #        super's phase-1 burst.
            #   ACT: [derf run (D table)] then [recip run (R table)] per
            #        super: 2 table loads per super, and derfs aren't
            #        blocked behind recips that wait on fresh DMA.
            def emit_phase1(c):
                r = emit_r_dma(c)
                return emit_p1_act(emit_p1_dve(c, r))

            def emit_phase2(c, t, m):
                emit_apply(c, t, m)
                emit_derf_pe(c, t)

            pending = [emit_phase1(0)]
            load_wb()
            load_z_quarter(0)
            pending += [emit_phase1(c) for c in range(1, K)]
            for s in range(NSUPER):
                for B in range(K * s, K * s + K):
                    emit_msg_cast(B, emit_msg_matmuls(B))
                if s + 1 < NSUPER:
                    load_z_quarter(s + 1)
                    nxt = [emit_phase1((s + 1) * K + k) for k in range(K)]
                else:
                    nxt = []
                for k in range(K):
                    emit_phase2(*pending[k])
                pending = nxt

            # ---- tail: PSUM -> SBUF fp32 -> HBM ----
            for h in (0, 1):
                o = outpool.tile([JT, ROWS], f32, tag=f"o{h}", name=f"o{h}")
                nc.vector.tensor_copy(o[:], acc[h][:])
                nc.sync.dma_start(outT_d[h * JT : (h + 1) * JT, :], o[:])

    _split_excess_waits(nc)
    return nc


def kernel(z, dist_matrix, W, B, _trace=False):
    from concourse.bass_utils import run_bass_kernel_spmd

    if _trace:
        _install_ntff_hook()

    dist = np.asarray(dist_matrix, np.float32)
    z = np.asarray(z, np.float32)
    W_np = np.asarray(W, np.float32)
    B_np = np.asarray(B, np.float32).reshape(1, D)
    has_bias = bool(np.any(B_np))

    key = ("nc", has_bias)
    if key not in _CACHE:
        _CACHE[key] = _build(has_bias)
    nc = _CACHE[key]

    # fp16 dist with an exact cutoff: values < 1 that round UP to 1.0
    # would flip the mask; pin them to the largest fp16 below 1.
    r16 = dist.astype(np.float16)
    bad = (dist < 1.0) & (r16 >= 1.0)
    if bad.any():
        r16[bad] = np.float16(0.99951171875)

    zT16 = np.ascontiguousarray(z.T.astype(np.float16))
    W16 = (W_np * WSCALE).astype(np.float16)
    B16 = (B_np * WSCALE).astype(np.float16)

    in_maps = []
    for c in range(NCORES):
        blk = np.ascontiguousarray(r16[c * ROWS : (c + 1) * ROWS, :].T)
        in_maps.append({"distT": blk, "zT": zT16, "w": W16, "b": B16})

    res = run_bass_kernel_spmd(
        nc, in_maps, core_ids=list(range(NCORES)), trace=_trace
    )
    _CACHE["last"] = res

    out = np.empty((N, D), np.float32)
    for c in range(NCORES):
        out[c * ROWS : (c + 1) * ROWS, :] = res.results[c]["outT"].T
    return out
